# revision 41
# baseline (speedup 1.0000x reference)
"""Trainium2 Bass kernel: GatedRecurrentCell (v9, all-fit).

Math (per batch b, channel i, time t):
    pa = x @ Wa^T (+ba via ACT bias) ; pi = x @ Wi^T (+bi via ACT bias)
    a  = sigmoid(gate) * 3**(-sigmoid(pa))
    c  = sqrt(1-a^2) * silu(pi)
    h_t = a_t*h_{t-1} + c_t   (h_{-1} = 0);  out = h

Design:
 1. 3**(-sigmoid(p)) == FA - FB*tanh(FC*p + FD) (5.5e-4 abs), so
    a = aA + nB*tanh(FC*pa + tb) with per-channel aA, nB.
 2. sqrt(1-a^2) ~ E + F*tanh(G*pa + H) for ALL channels (per-channel
    params from a pdf-weighted table refit; end-to-end rel err 7.8e-3,
    better than the old exact-sqrt split at 8.7e-3). This removes the
    Square/Sqrt passes entirely, so ACT uses ONE table set
    (silu_and_others: silu+tanh) -> zero table switches.
 3. q = qF*tq + qE via DVE tensor_scalar (bf16 fast mode), then
    c = q*w via DVE tensor_tensor (bf16 2x).
 4. Per chunk (PE order pinned): pa GEMM first, into TWO [128,1024]
    PSUM tiles (2 banks each) -- PSUM accumulation tiles are
    dependency-tracked as one unit, so a single 2048-wide tile forces
    every tanh to wait for the full GEMM (1.5us/chunk PE stall); the
    split lets the h0 tanhs run mid-GEMM. Then pi GEMM -> [128,2048]
    PSUM (4 banks), one silu@2048. ACT queue order pinned
    (tq_h0, th_h0, tq_h1, th_h1, silu).
 5. a-affine (a = nB*th + aA) on DVE in bf16 (tensor_scalar 4x mode,
    ~0.8us/chunk); th/a/q/c all bf16 (adds nothing visible to the
    error). GPSIMD only does seam memsets -- it otherwise contends
    with DVE for the shared SBUF port and slows fast-mode DVE ops ~3x.
 6. Recurrence: fp32 tensor_tensor_scan on DVE over pairs of chunks
    (a at the pair seam is zeroed); last 2 chunks single; the final
    chunk runs silu/c/scan/DMA at half granularity with a chained
    scan initial for the shortest tail.
 7. Startup: xT m0-quarters DMAd from the gpsimd queue in parallel
    with weights on the sync queue; dummy warm-up matmuls on wi0 warm
    the PE HAM clock-gate during the DMA wait; a dummy 1-col Silu
    triggers the ACT table load immediately (one table set total).

Mapping: data-parallel over batch (8 cores, 1 batch each); channels on
partitions (16 chunks of 128), time on the free dim. GEMMs in bf16.
"""

import functools
import os

import numpy as np

B, S, D, I = 8, 2048, 512, 2048
P = 128
NCORES = 8

# fit of 3^(-sigmoid(p)) = FA - FB*tanh(FC*p + FD), max abs err 5.5e-4
FA = 0.66661083
FB = 0.33324857
FC = 0.5096609
FD = 0.27426951

# knobs
AFF_DVE_N = int(os.environ.get("GRC_AFF_DVE", "16"))  # a-affines moved to DVE
CMUL_GP_N = int(os.environ.get("GRC_CMUL_GP", "0"))  # c=q*w on GPSIMD for
# the first N chunks (GPSIMD is otherwise idle; tail chunks stay on DVE so
# their scans aren't delayed by GPSIMD's slower elementwise rate)
SCAN_GP = os.environ.get("GRC_SCAN_GP", "")  # comma group idxs on GPSIMD

# per-alpha params of sqrt(1 - (alpha 3^-sigmoid(p))^2) ~ E + F*tanh(G*p+H)
# (pdf-weighted least-squares refit; valid for the full alpha range here)
QFIT_ALPHAS = [0.88000000, 0.88199333, 0.88398667, 0.88598000, 0.88797333, 0.88996667, 0.89196000, 0.89395333, 0.89594667, 0.89794000, 0.89993333, 0.90192667, 0.90392000, 0.90591333, 0.90790667, 0.90990000, 0.91189333, 0.91388667, 0.91588000, 0.91787333, 0.91986667, 0.92186000, 0.92385333, 0.92584667, 0.92784000, 0.92983333, 0.93182667, 0.93382000, 0.93581333, 0.93780667, 0.93980000, 0.94179333, 0.94378667, 0.94578000, 0.94777333, 0.94976667, 0.95176000, 0.95375333, 0.95574667, 0.95774000, 0.95973333, 0.96172667, 0.96372000, 0.96571333, 0.96770667, 0.96970000, 0.97169333, 0.97368667, 0.97568000, 0.97767333, 0.97966667, 0.98166000, 0.98365333, 0.98564667, 0.98764000, 0.98963333, 0.99162667, 0.99362000, 0.99561333, 0.99760667, 0.99960000]
QFIT_E = [0.71933143, 0.71742518, 0.71550108, 0.71355876, 0.71159781, 0.70961777, 0.70761823, 0.70559871, 0.70355872, 0.70149777, 0.69941532, 0.69731082, 0.69518371, 0.69303337, 0.69085916, 0.68866043, 0.6864365, 0.6841866, 0.68190999, 0.67960586, 0.67727333, 0.67491154, 0.67251952, 0.67009625, 0.6676407, 0.66515172, 0.6626281, 0.66006859, 0.65747183, 0.65483635, 0.65216063, 0.64944301, 0.64668169, 0.64387479, 0.64102025, 0.63811582, 0.63515915, 0.6321476, 0.62907833, 0.62594828, 0.62275404, 0.61949189, 0.61615776, 0.6127471, 0.60925485, 0.60567543, 0.60200249, 0.59822889, 0.59434658, 0.59034627, 0.58621727, 0.58194721, 0.57752149, 0.5729228, 0.56813033, 0.56311861, 0.55785594, 0.55230202, 0.54640405, 0.54009043, 0.53325945]
QFIT_F = [0.23774414, 0.23947298, 0.24121995, 0.24298545, 0.24476994, 0.24657388, 0.24839774, 0.25024203, 0.25210728, 0.25399401, 0.25590282, 0.25783429, 0.25978905, 0.26176774, 0.26377108, 0.26579975, 0.26785453, 0.26993621, 0.27204563, 0.27418366, 0.27635126, 0.27854937, 0.28077905, 0.28304142, 0.28533761, 0.28766887, 0.29003652, 0.29244195, 0.29488666, 0.29737224, 0.29990039, 0.30247294, 0.30509185, 0.30775923, 0.31047734, 0.31324867, 0.31607585, 0.31896179, 0.32190964, 0.32492283, 0.32800514, 0.33116073, 0.33439414, 0.33771047, 0.34111538, 0.34461515, 0.34821689, 0.35192864, 0.35575949, 0.35971991, 0.36382199, 0.36807975, 0.37250975, 0.37713169, 0.38196927, 0.38705155, 0.39241485, 0.39810538, 0.40418387, 0.41073294, 0.4178704]
QFIT_G = [0.53053654, 0.53025385, 0.52996473, 0.52966898, 0.52936639, 0.52905672, 0.52873975, 0.52841521, 0.52808285, 0.5277424, 0.52739356, 0.52703604, 0.52666952, 0.52629366, 0.5259081, 0.5255125, 0.52510644, 0.52468951, 0.52426128, 0.52382129, 0.52336904, 0.52290402, 0.52242568, 0.52193342, 0.52142663, 0.52090464, 0.52036673, 0.51981216, 0.5192401, 0.51864969, 0.51803998, 0.51740998, 0.5167586, 0.51608465, 0.51538688, 0.5146639, 0.51391421, 0.51313618, 0.512328, 0.51148774, 0.51061321, 0.50970204, 0.50875159, 0.5077589, 0.50672069, 0.50563327, 0.50449247, 0.50329355, 0.5020311, 0.50069891, 0.49928974, 0.49779515, 0.49620515, 0.49450777, 0.49268853, 0.49072957, 0.48860845, 0.48629636, 0.48375515, 0.48093245, 0.47775255]
QFIT_H = [0.6873514, 0.68859901, 0.68986176, 0.69114, 0.69243407, 0.69374437, 0.69507125, 0.69641513, 0.69777642, 0.69915554, 0.70055296, 0.70196914, 0.70340457, 0.70485975, 0.70633523, 0.70783155, 0.7093493, 0.71088909, 0.71245156, 0.71403737, 0.71564725, 0.71728191, 0.71894214, 0.72062877, 0.72234265, 0.72408469, 0.72585588, 0.72765722, 0.72948979, 0.73135477, 0.73325335, 0.73518684, 0.73715666, 0.73916427, 0.74121127, 0.74329939, 0.74543044, 0.74760642, 0.7498295, 0.75210198, 0.75442641, 0.75680556, 0.75924244, 0.76174039, 0.76430309, 0.76693459, 0.76963943, 0.77242271, 0.77529012, 0.77824814, 0.78130421, 0.78446682, 0.78774591, 0.79115314, 0.79470237, 0.79841037, 0.80229777, 0.80639043, 0.81072176, 0.81533639, 0.82029642]

CONST_NAMES = ["aA", "nB", "tb", "sb", "qs", "qb", "qE", "qF"]


def _build_nc(s, d, i, nfit=0, silu=True):
    import concourse.bacc as bacc
    import concourse.mybir as mybir
    import concourse.tile as tile
    from concourse.tile import add_dep_helper
    from contextlib import ExitStack

    F32 = mybir.dt.float32
    BF16 = mybir.dt.bfloat16
    AF = mybir.ActivationFunctionType
    ALU = mybir.AluOpType

    nd = d // P            # contraction chunks (128 rows each)
    ni = i // P            # channel chunks
    nmm = s // 512         # matmuls (N=512) per GEMM
    half = s // 2          # tanh granularity

    aff_dve = set()
    if AFF_DVE_N > 0:
        aff_dve = {ni - 1 - j for j in range(min(AFF_DVE_N, ni))}
    scan_gp = {int(t) for t in SCAN_GP.split(",") if t.strip() != ""}

    def pair_groups(ics, singles_at_end=2):
        ics = list(ics)
        nsing = singles_at_end if len(ics) >= 4 else len(ics) % 2
        body = ics[:len(ics) - nsing] if nsing else ics
        gs = [body[j:j + 2] for j in range(0, len(body), 2)]
        gs += [[ic] for ic in ics[len(ics) - nsing:]] if nsing else []
        return gs

    if ni >= 4:
        # single chunk FIRST (its half-scans start ~7us earlier, pulling
        # the whole saturated-DVE schedule forward) and single LAST (short
        # tail); pairs in between
        ics = list(range(ni))
        nlead = 4 if ni >= 8 else 2
        ntrail = 4 if ni >= 12 else 2
        groups = ([[ics[j]] for j in range(nlead)]
                  + [ics[j:j + 2] for j in range(nlead, ni - ntrail, 2)]
                  + [[ics[j]] for j in range(ni - ntrail, ni)])
    else:
        groups = pair_groups(range(ni), singles_at_end=2)

    nc = bacc.Bacc("TRN2", target_bir_lowering=False, debug=False,
                   num_devices=NCORES)

    xT_d = nc.dram_tensor("xT", [d, s], BF16, kind="ExternalInput").ap()
    waT_d = nc.dram_tensor("WaT", [ni, P, d], BF16, kind="ExternalInput").ap()
    wiT_d = nc.dram_tensor("WiT", [ni, P, d], BF16, kind="ExternalInput").ap()
    cst_d = nc.dram_tensor("csts", [P, len(CONST_NAMES) * ni], F32,
                           kind="ExternalInput").ap()
    out_d = nc.dram_tensor("out", [i, s], BF16, kind="ExternalOutput").ap()

    with tile.TileContext(nc) as tc:
        with ExitStack() as ctx:
            const_pool = ctx.enter_context(tc.tile_pool(name="const", bufs=1))
            xt_pool = ctx.enter_context(tc.tile_pool(name="xt", bufs=1))
            wst_pool = ctx.enter_context(tc.tile_pool(name="wst", bufs=1))
            ps_pool = ctx.enter_context(
                tc.tile_pool(name="mmpsum", bufs=1, space="PSUM"))
            rows = ctx.enter_context(tc.tile_pool(name="rows", bufs=1))

            # ---- weights for chunk 0 first (sync queue) -----------------
            w_sbs = {}

            def load_weights(ic):
                wi_sb = wst_pool.tile([P, d], BF16, name=f"wi{ic}", tag="wi",
                                      bufs=3)
                wa_sb = wst_pool.tile([P, d], BF16, name=f"wa{ic}", tag="wa",
                                      bufs=3)
                nc.sync.dma_start(wi_sb[:], wiT_d[ic])
                nc.sync.dma_start(wa_sb[:], waT_d[ic])
                w_sbs[ic] = (wi_sb, wa_sb)

            load_weights(0)

            # ---- x stream: m0 quarters from the gpsimd queue ------------
            xT_sb = [xt_pool.tile([P, s], BF16, name=f"xT{k}")
                     for k in range(nd)]
            q = 512
            for k in range(nd):
                nc.gpsimd.dma_start(xT_sb[k][:, 0:q],
                                    xT_d[k * P:(k + 1) * P, 0:q])

            # consts (one small DMA on sync queue)
            cst_t = const_pool.tile([P, len(CONST_NAMES) * ni], F32,
                                    name="cst")
            nc.sync.dma_start(cst_t[:], cst_d[:])

            def cc(nm, ic):
                base = CONST_NAMES.index(nm) * ni
                return cst_t[:, base + ic:base + ic + 1]

            # pinned program order on the ACT and PE queues: the Tile
            # scheduler otherwise interleaves silu between the tanh halves
            # and flips the pa/pi GEMM order, stalling the PE ~3.3us/chunk.
            act_chain = []

            def act(out_ap, in_ap, func, **kw):
                inst = nc.scalar.activation(out_ap, in_ap, func, **kw)
                if act_chain:
                    add_dep_helper(inst.ins, act_chain[-1].ins, False,
                                   "act order")
                act_chain.append(inst)
                return inst

            mm_chain = []

            def mm(out_ap, lhs_ap, rhs_ap, **kw):
                inst = nc.tensor.matmul(out_ap, lhs_ap, rhs_ap, **kw)
                if mm_chain:
                    add_dep_helper(inst.ins, mm_chain[-1].ins, False,
                                   "pe order")
                mm_chain.append(inst)
                return inst

            # ---- ACT table preload + PE warm-up during the DMA wait -----
            dum = rows.tile([P, 8], F32, name="dum")
            if silu:
                act(dum[:, 0:1], cc("sb", 0), AF.Silu)
            else:
                act(dum[:, 0:1], cc("sb", 0), AF.Sigmoid)
            act(dum[:, 1:2], cc("tb", 0), AF.Tanh)
            wi0 = w_sbs[0][0]
            wn = min(512, d)
            # warm-up MMs write into the pi-tag PSUM tile (reused by the
            # first real pi GEMM afterwards) so PSUM stays within 8 banks.
            warm_ps = ps_pool.tile([P, s], F32, name="warm", tag="pi",
                                   bufs=1)
            for _ in range(8 if s >= 2048 else 1):
                mm(warm_ps[:, 0:wn], wi0[:, 0:P], wi0[:, 0:wn],
                   start=True, stop=True)

            # remaining x quarters (m1..): gpsimd queue, m-major
            for m in range(1, s // q):
                for k in range(nd):
                    nc.gpsimd.dma_start(
                        xT_sb[k][:, m * q:(m + 1) * q],
                        xT_d[k * P:(k + 1) * P, m * q:(m + 1) * q])
            load_weights(1)

            def gemm(ps, w_sb):
                # m-outer, k-inner: each 512-col slice finishes ASAP
                for m in range(nmm):
                    lo = m * 512
                    for k in range(nd):
                        mm(ps[:, lo:lo + 512],
                           w_sb[:, k * P:(k + 1) * P],
                           xT_sb[k][:, lo:lo + 512],
                           start=(k == 0), stop=(k == nd - 1))

            def gemm_pa_split(ic, wa_sb):
                """pa GEMM into two [P, half] PSUM tiles (2 banks each)."""
                if nmm >= 2:
                    pa0 = ps_pool.tile([P, half], F32, name=f"pa0{ic}",
                                       tag="pa0", bufs=1)
                    pa1 = ps_pool.tile([P, half], F32, name=f"pa1{ic}",
                                       tag="pa1", bufs=1)
                    for m in range(nmm):
                        ps = pa0 if m < nmm // 2 else pa1
                        lo_t = (m % (nmm // 2)) * 512
                        lo = m * 512
                        for k in range(nd):
                            mm(ps[:, lo_t:lo_t + 512],
                               wa_sb[:, k * P:(k + 1) * P],
                               xT_sb[k][:, lo:lo + 512],
                               start=(k == 0), stop=(k == nd - 1))
                    return [pa0, pa1]
                pa0 = ps_pool.tile([P, s], F32, name=f"pa{ic}", tag="pa0",
                                   bufs=1)
                gemm(pa0, wa_sb)
                return [pa0[:, 0:half], pa0[:, half:s]]

            def chunk_front(ic, ap_t, jslot):
                """pa GEMM -> tq halves + th@full; pi GEMM -> silu@full;
                a-affine on GPSIMD. Returns (w_t, tq_t)."""
                if ic not in w_sbs:
                    load_weights(ic)
                wi_sb, wa_sb = w_sbs.pop(ic)

                # pa GEMM first, into TWO half-width PSUM tiles so the
                # h0 tanhs can start mid-GEMM (PSUM accumulation tiles
                # are dependency-tracked as one unit).
                pa_ps = gemm_pa_split(ic, wa_sb)
                tq_t = rows.tile([P, s], BF16, name=f"tq{ic}", tag="tq",
                                 bufs=3)
                th_t = rows.tile([P, s], BF16, name=f"th{ic}", tag="th",
                                 bufs=3)
                for hh in range(2):
                    sl = slice(hh * half, (hh + 1) * half)
                    act(tq_t[:, sl], pa_ps[hh][:], AF.Tanh,
                        scale=cc("qs", ic), bias=cc("qb", ic))
                    act(th_t[:, sl], pa_ps[hh][:], AF.Tanh,
                        scale=FC, bias=cc("tb", ic))

                # pi GEMM -> one 2048-wide PSUM tile -> single silu
                pi_ps = ps_pool.tile([P, s], F32, name=f"pi{ic}", tag="pi",
                                     bufs=1)
                gemm(pi_ps, wi_sb)
                w_t = rows.tile([P, s], BF16, name=f"w{ic}", tag="w", bufs=2)
                if silu:
                    act(w_t[:], pi_ps[:], AF.Silu, bias=cc("sb", ic))
                else:
                    sg = rows.tile([P, s], F32, name=f"sg{ic}", tag="sg",
                                   bufs=2)
                    act(sg[:], pi_ps[:], AF.Sigmoid, bias=cc("sb", ic))
                    pib = rows.tile([P, s], F32, name=f"pib{ic}", tag="pib",
                                    bufs=2)
                    act(pib[:], pi_ps[:], AF.Identity, bias=cc("sb", ic))
                    nc.vector.tensor_mul(w_t[:], sg[:], pib[:])

                return w_t, tq_t, th_t

            def alloc_pair(g, tag_p, tag_s, dtype, bufs_p, bufs_s):
                if len(g) == 2:
                    return rows.tile([P, 2 * s], dtype, name=f"{tag_p}{g[0]}",
                                     tag=tag_p, bufs=bufs_p)
                return rows.tile([P, s], dtype, name=f"{tag_s}{g[0]}",
                                 tag=tag_s, bufs=bufs_s)

            def scan_group(g, ap_t, cp_t, eng):
                w2 = len(g) * s
                h_t = rows.tile([P, w2], BF16, name=f"h{g[0]}",
                                tag="hp" if len(g) == 2 else "hs", bufs=2)
                eng.tensor_tensor_scan(
                    h_t[:], ap_t[:], cp_t[:], 0.0,
                    op0=ALU.mult, op1=ALU.add)
                for j, ic in enumerate(g):
                    nc.sync.dma_start(out_d[ic * P:(ic + 1) * P, :],
                                      h_t[:, j * s:(j + 1) * s])

            def last_single(ic, nparts=2):
                """Final chunk: affine+q hoisted before the pi GEMM; silu,
                c, scan, DMA at half granularity for the shortest tail."""
                if ic not in w_sbs:
                    load_weights(ic)
                wi_sb, wa_sb = w_sbs.pop(ic)
                pa_ps = gemm_pa_split(ic, wa_sb)
                tq_t = rows.tile([P, s], BF16, name=f"tq{ic}", tag="tq",
                                 bufs=3)
                th_t = rows.tile([P, s], BF16, name=f"th{ic}", tag="th",
                                 bufs=3)
                for hh in range(2):
                    sl = slice(hh * half, (hh + 1) * half)
                    act(tq_t[:, sl], pa_ps[hh][:], AF.Tanh,
                        scale=cc("qs", ic), bias=cc("qb", ic))
                    act(th_t[:, sl], pa_ps[hh][:], AF.Tanh,
                        scale=FC, bias=cc("tb", ic))
                ap_t = rows.tile([P, s], BF16, name=f"as{ic}", tag="as",
                                 bufs=2)
                nc.vector.tensor_scalar(ap_t[:], th_t[:], cc("nB", ic),
                                        cc("aA", ic), op0=ALU.mult,
                                        op1=ALU.add)
                q_t = rows.tile([P, s], BF16, name=f"q{ic}", tag="q", bufs=2)
                nc.vector.tensor_scalar(q_t[:], tq_t[:], cc("qF", ic),
                                        cc("qE", ic), op0=ALU.mult,
                                        op1=ALU.add)

                pi_ps = ps_pool.tile([P, s], F32, name=f"pi{ic}", tag="pi",
                                     bufs=1)
                gemm(pi_ps, wi_sb)
                w_t = rows.tile([P, s], BF16, name=f"w{ic}", tag="w", bufs=2)
                cp_t = rows.tile([P, s], BF16, name=f"cs{ic}", tag="cs",
                                 bufs=2)
                h_t = rows.tile([P, s], BF16, name=f"h{ic}", tag="hs",
                                bufs=2)
                qw = s // nparts
                for hh in range(nparts):
                    sl = slice(hh * qw, (hh + 1) * qw)
                    if silu:
                        act(w_t[:, sl], pi_ps[:, sl], AF.Silu,
                            bias=cc("sb", ic))
                    else:
                        sg = rows.tile([P, qw], F32, name=f"sg{ic}_{hh}",
                                       tag="sg", bufs=2)
                        act(sg[:], pi_ps[:, sl], AF.Sigmoid,
                            bias=cc("sb", ic))
                        pib = rows.tile([P, qw], F32, name=f"pib{ic}_{hh}",
                                        tag="pib", bufs=2)
                        act(pib[:], pi_ps[:, sl], AF.Identity,
                            bias=cc("sb", ic))
                        nc.vector.tensor_mul(w_t[:, sl], sg[:], pib[:])
                    nc.vector.tensor_mul(cp_t[:, sl], q_t[:, sl],
                                         w_t[:, sl])
                    nc.vector.tensor_tensor_scan(
                        h_t[:, sl], ap_t[:, sl], cp_t[:, sl],
                        0.0 if hh == 0 else h_t[:, hh * qw - 1:hh * qw],
                        op0=ALU.mult, op1=ALU.add)
                    nc.sync.dma_start(out_d[ic * P:(ic + 1) * P, sl],
                                      h_t[:, sl])

            for gi, g in enumerate(groups):
                if len(g) == 1:
                    last_single(g[0])
                    continue
                on_gp = gi in scan_gp
                ap_t = alloc_pair(g, "ap", "as", BF16, 2, 2)
                cp_t = alloc_pair(g, "cp", "cs", BF16, 2, 2)
                for j, ic in enumerate(g):
                    if on_gp:
                        aff_dve.add(ic)  # keep GPSIMD free for its scan
                    w_t, tq_t, th_t = chunk_front(ic, ap_t, j)
                    # DVE order q -> aff -> c: q's input (tq) is ready
                    # before th, so the FIFO head-blocks less
                    q_t = rows.tile([P, s], BF16, name=f"q{ic}", tag="q",
                                    bufs=2)
                    nc.vector.tensor_scalar(q_t[:], tq_t[:], cc("qF", ic),
                                            cc("qE", ic), op0=ALU.mult,
                                            op1=ALU.add)
                    a_v = ap_t[:, j * s:(j + 1) * s]
                    aff_eng = nc.vector if ic in aff_dve else nc.gpsimd
                    aff_eng.tensor_scalar(a_v, th_t[:], cc("nB", ic),
                                          cc("aA", ic), op0=ALU.mult,
                                          op1=ALU.add)
                    if j == 1:
                        nc.gpsimd.memset(ap_t[:, s:s + 1], 0.0)
                    cm_eng = nc.gpsimd if ic < CMUL_GP_N else nc.vector
                    cm_eng.tensor_mul(cp_t[:, j * s:(j + 1) * s], q_t[:],
                                      w_t[:])
                scan_group(g, ap_t, cp_t,
                           nc.gpsimd if on_gp else nc.vector)

    nc.compile()
    return nc


@functools.lru_cache(maxsize=4)
def _get_nc(s=S, d=D, i=I, nfit=0):
    return _build_nc(s, d, i, nfit=nfit)


LAST_RESULTS = None


def _prep_core_inputs(xb, shared):
    import ml_dtypes
    xT = np.ascontiguousarray(xb.T).astype(ml_dtypes.bfloat16)
    m = {"xT": xT}
    m.update(shared)
    return m


def _prep_shared(Wa, ba, Wi, bi, gate, d, i):
    """Sort channels by alpha, build device inputs. Returns
    (shared dict, nfit, perm, out_scale[i])."""
    import ml_dtypes
    ni = i // P
    nd = d // P
    alpha_u = 1.0 / (1.0 + np.exp(-gate.astype(np.float64)))
    perm = np.argsort(alpha_u, kind="stable")
    Wa = Wa[perm]
    Wi = Wi[perm]
    ba = ba[perm]
    bi = bi[perm]
    alpha = alpha_u[perm]

    WaT = np.ascontiguousarray(
        Wa.reshape(ni, P, nd, P).transpose(0, 3, 2, 1).reshape(ni, P, d)
    ).astype(ml_dtypes.bfloat16)
    WiT = np.ascontiguousarray(
        Wi.reshape(ni, P, nd, P).transpose(0, 3, 2, 1).reshape(ni, P, d)
    ).astype(ml_dtypes.bfloat16)

    aA = (alpha * FA).astype(np.float32)
    nB = (-alpha * FB).astype(np.float32)
    tb = (FC * ba.astype(np.float64) + FD).astype(np.float32)
    sb = bi.astype(np.float32)

    al = np.clip(alpha, QFIT_ALPHAS[0], QFIT_ALPHAS[-1])
    E = np.interp(al, QFIT_ALPHAS, QFIT_E)
    F = np.interp(al, QFIT_ALPHAS, QFIT_F)
    G = np.interp(al, QFIT_ALPHAS, QFIT_G)
    H = np.interp(al, QFIT_ALPHAS, QFIT_H)
    qs = G.astype(np.float32)
    qb = (G * ba.astype(np.float64) + H).astype(np.float32)
    scale = np.ones(i, np.float64)

    def vec(v):
        return np.ascontiguousarray(v.astype(np.float32).reshape(ni, P).T)

    vals = {"aA": aA, "nB": nB, "tb": tb, "sb": sb, "qs": qs, "qb": qb,
            "qE": E.astype(np.float32), "qF": F.astype(np.float32)}
    csts = np.concatenate([vec(vals[nm]) for nm in CONST_NAMES], axis=1)
    shared = {"WaT": WaT, "WiT": WiT, "csts": np.ascontiguousarray(csts)}
    return shared, ni, perm, scale.astype(np.float32)


def kernel(x, Wa, ba, Wi, bi, gate):
    global LAST_RESULTS
    from concourse.bass_utils import run_bass_kernel_spmd

    x = np.asarray(x, dtype=np.float32)
    b, s, d = x.shape
    i = Wa.shape[0]

    shared, nfit, perm, oscale = _prep_shared(
        np.asarray(Wa, np.float32), np.asarray(ba, np.float32),
        np.asarray(Wi, np.float32), np.asarray(bi, np.float32),
        np.asarray(gate, np.float32), d, i)
    nc = _get_nc(s, d, i, 0)

    in_maps = [_prep_core_inputs(x[bb], shared) for bb in range(b)]
    res = run_bass_kernel_spmd(nc, in_maps, list(range(b)))
    LAST_RESULTS = res
    out = np.empty((b, s, i), np.float32)
    for bb in range(b):
        hs = np.asarray(res.results[bb]["out"]).astype(np.float32).T * oscale
        out[bb, :, perm] = hs.T
    return out


# revision 42
# speedup vs baseline: 1.0076x; 1.0076x over previous
"""Trainium2 Bass kernel: GatedRecurrentCell (v9, all-fit).

Math (per batch b, channel i, time t):
    pa = x @ Wa^T (+ba via ACT bias) ; pi = x @ Wi^T (+bi via ACT bias)
    a  = sigmoid(gate) * 3**(-sigmoid(pa))
    c  = sqrt(1-a^2) * silu(pi)
    h_t = a_t*h_{t-1} + c_t   (h_{-1} = 0);  out = h

Design:
 1. 3**(-sigmoid(p)) == FA - FB*tanh(FC*p + FD) (5.5e-4 abs), so
    a = aA + nB*tanh(FC*pa + tb) with per-channel aA, nB.
 2. sqrt(1-a^2) ~ E + F*tanh(G*pa + H) for ALL channels (per-channel
    params from a pdf-weighted table refit; end-to-end rel err 7.8e-3,
    better than the old exact-sqrt split at 8.7e-3). This removes the
    Square/Sqrt passes entirely, so ACT uses ONE table set
    (silu_and_others: silu+tanh) -> zero table switches.
 3. q = qF*tq + qE via DVE tensor_scalar (bf16 fast mode), then
    c = q*w via DVE tensor_tensor (bf16 2x).
 4. Per chunk (PE order pinned): pa GEMM first, into TWO [128,1024]
    PSUM tiles (2 banks each) -- PSUM accumulation tiles are
    dependency-tracked as one unit, so a single 2048-wide tile forces
    every tanh to wait for the full GEMM (1.5us/chunk PE stall); the
    split lets the h0 tanhs run mid-GEMM. Then pi GEMM -> [128,2048]
    PSUM (4 banks), one silu@2048. ACT queue order pinned
    (tq_h0, th_h0, tq_h1, th_h1, silu).
 5. a-affine (a = nB*th + aA) on DVE in bf16 (tensor_scalar 4x mode,
    ~0.8us/chunk); th/a/q/c all bf16 (adds nothing visible to the
    error). GPSIMD only does seam memsets -- it otherwise contends
    with DVE for the shared SBUF port and slows fast-mode DVE ops ~3x.
 6. Recurrence: fp32 tensor_tensor_scan on DVE over pairs of chunks
    (a at the pair seam is zeroed); last 2 chunks single; the final
    chunk runs silu/c/scan/DMA at half granularity with a chained
    scan initial for the shortest tail.
 7. Startup: xT m0-quarters DMAd from the gpsimd queue in parallel
    with weights on the sync queue; dummy warm-up matmuls on wi0 warm
    the PE HAM clock-gate during the DMA wait; a dummy 1-col Silu
    triggers the ACT table load immediately (one table set total).

Mapping: data-parallel over batch (8 cores, 1 batch each); channels on
partitions (16 chunks of 128), time on the free dim. GEMMs in bf16.
"""

import functools
import os

import numpy as np

B, S, D, I = 8, 2048, 512, 2048
P = 128
NCORES = 8

# fit of 3^(-sigmoid(p)) = FA - FB*tanh(FC*p + FD), max abs err 5.5e-4
FA = 0.66661083
FB = 0.33324857
FC = 0.5096609
FD = 0.27426951

# knobs
AFF_DVE_N = int(os.environ.get("GRC_AFF_DVE", "16"))  # a-affines moved to DVE
CMUL_GP_N = int(os.environ.get("GRC_CMUL_GP", "0"))  # c=q*w on GPSIMD for
# the first N chunks (GPSIMD is otherwise idle; tail chunks stay on DVE so
# their scans aren't delayed by GPSIMD's slower elementwise rate)
SCAN_GP = os.environ.get("GRC_SCAN_GP", "")  # comma group idxs on GPSIMD

# per-alpha params of sqrt(1 - (alpha 3^-sigmoid(p))^2) ~ E + F*tanh(G*p+H)
# (pdf-weighted least-squares refit; valid for the full alpha range here)
QFIT_ALPHAS = [0.88000000, 0.88199333, 0.88398667, 0.88598000, 0.88797333, 0.88996667, 0.89196000, 0.89395333, 0.89594667, 0.89794000, 0.89993333, 0.90192667, 0.90392000, 0.90591333, 0.90790667, 0.90990000, 0.91189333, 0.91388667, 0.91588000, 0.91787333, 0.91986667, 0.92186000, 0.92385333, 0.92584667, 0.92784000, 0.92983333, 0.93182667, 0.93382000, 0.93581333, 0.93780667, 0.93980000, 0.94179333, 0.94378667, 0.94578000, 0.94777333, 0.94976667, 0.95176000, 0.95375333, 0.95574667, 0.95774000, 0.95973333, 0.96172667, 0.96372000, 0.96571333, 0.96770667, 0.96970000, 0.97169333, 0.97368667, 0.97568000, 0.97767333, 0.97966667, 0.98166000, 0.98365333, 0.98564667, 0.98764000, 0.98963333, 0.99162667, 0.99362000, 0.99561333, 0.99760667, 0.99960000]
QFIT_E = [0.71933143, 0.71742518, 0.71550108, 0.71355876, 0.71159781, 0.70961777, 0.70761823, 0.70559871, 0.70355872, 0.70149777, 0.69941532, 0.69731082, 0.69518371, 0.69303337, 0.69085916, 0.68866043, 0.6864365, 0.6841866, 0.68190999, 0.67960586, 0.67727333, 0.67491154, 0.67251952, 0.67009625, 0.6676407, 0.66515172, 0.6626281, 0.66006859, 0.65747183, 0.65483635, 0.65216063, 0.64944301, 0.64668169, 0.64387479, 0.64102025, 0.63811582, 0.63515915, 0.6321476, 0.62907833, 0.62594828, 0.62275404, 0.61949189, 0.61615776, 0.6127471, 0.60925485, 0.60567543, 0.60200249, 0.59822889, 0.59434658, 0.59034627, 0.58621727, 0.58194721, 0.57752149, 0.5729228, 0.56813033, 0.56311861, 0.55785594, 0.55230202, 0.54640405, 0.54009043, 0.53325945]
QFIT_F = [0.23774414, 0.23947298, 0.24121995, 0.24298545, 0.24476994, 0.24657388, 0.24839774, 0.25024203, 0.25210728, 0.25399401, 0.25590282, 0.25783429, 0.25978905, 0.26176774, 0.26377108, 0.26579975, 0.26785453, 0.26993621, 0.27204563, 0.27418366, 0.27635126, 0.27854937, 0.28077905, 0.28304142, 0.28533761, 0.28766887, 0.29003652, 0.29244195, 0.29488666, 0.29737224, 0.29990039, 0.30247294, 0.30509185, 0.30775923, 0.31047734, 0.31324867, 0.31607585, 0.31896179, 0.32190964, 0.32492283, 0.32800514, 0.33116073, 0.33439414, 0.33771047, 0.34111538, 0.34461515, 0.34821689, 0.35192864, 0.35575949, 0.35971991, 0.36382199, 0.36807975, 0.37250975, 0.37713169, 0.38196927, 0.38705155, 0.39241485, 0.39810538, 0.40418387, 0.41073294, 0.4178704]
QFIT_G = [0.53053654, 0.53025385, 0.52996473, 0.52966898, 0.52936639, 0.52905672, 0.52873975, 0.52841521, 0.52808285, 0.5277424, 0.52739356, 0.52703604, 0.52666952, 0.52629366, 0.5259081, 0.5255125, 0.52510644, 0.52468951, 0.52426128, 0.52382129, 0.52336904, 0.52290402, 0.52242568, 0.52193342, 0.52142663, 0.52090464, 0.52036673, 0.51981216, 0.5192401, 0.51864969, 0.51803998, 0.51740998, 0.5167586, 0.51608465, 0.51538688, 0.5146639, 0.51391421, 0.51313618, 0.512328, 0.51148774, 0.51061321, 0.50970204, 0.50875159, 0.5077589, 0.50672069, 0.50563327, 0.50449247, 0.50329355, 0.5020311, 0.50069891, 0.49928974, 0.49779515, 0.49620515, 0.49450777, 0.49268853, 0.49072957, 0.48860845, 0.48629636, 0.48375515, 0.48093245, 0.47775255]
QFIT_H = [0.6873514, 0.68859901, 0.68986176, 0.69114, 0.69243407, 0.69374437, 0.69507125, 0.69641513, 0.69777642, 0.69915554, 0.70055296, 0.70196914, 0.70340457, 0.70485975, 0.70633523, 0.70783155, 0.7093493, 0.71088909, 0.71245156, 0.71403737, 0.71564725, 0.71728191, 0.71894214, 0.72062877, 0.72234265, 0.72408469, 0.72585588, 0.72765722, 0.72948979, 0.73135477, 0.73325335, 0.73518684, 0.73715666, 0.73916427, 0.74121127, 0.74329939, 0.74543044, 0.74760642, 0.7498295, 0.75210198, 0.75442641, 0.75680556, 0.75924244, 0.76174039, 0.76430309, 0.76693459, 0.76963943, 0.77242271, 0.77529012, 0.77824814, 0.78130421, 0.78446682, 0.78774591, 0.79115314, 0.79470237, 0.79841037, 0.80229777, 0.80639043, 0.81072176, 0.81533639, 0.82029642]

CONST_NAMES = ["aA", "nB", "tb", "sb", "qs", "qb", "qE", "qF"]


def _build_nc(s, d, i, nfit=0, silu=True):
    import concourse.bacc as bacc
    import concourse.mybir as mybir
    import concourse.tile as tile
    from concourse.tile import add_dep_helper
    from contextlib import ExitStack

    F32 = mybir.dt.float32
    BF16 = mybir.dt.bfloat16
    AF = mybir.ActivationFunctionType
    ALU = mybir.AluOpType

    nd = d // P            # contraction chunks (128 rows each)
    ni = i // P            # channel chunks
    nmm = s // 512         # matmuls (N=512) per GEMM
    half = s // 2          # tanh granularity

    aff_dve = set()
    if AFF_DVE_N > 0:
        aff_dve = {ni - 1 - j for j in range(min(AFF_DVE_N, ni))}
    scan_gp = {int(t) for t in SCAN_GP.split(",") if t.strip() != ""}

    def pair_groups(ics, singles_at_end=2):
        ics = list(ics)
        nsing = singles_at_end if len(ics) >= 4 else len(ics) % 2
        body = ics[:len(ics) - nsing] if nsing else ics
        gs = [body[j:j + 2] for j in range(0, len(body), 2)]
        gs += [[ic] for ic in ics[len(ics) - nsing:]] if nsing else []
        return gs

    if ni >= 4:
        # single chunk FIRST (its half-scans start ~7us earlier, pulling
        # the whole saturated-DVE schedule forward) and single LAST (short
        # tail); pairs in between
        ics = list(range(ni))
        nlead = 4 if ni >= 8 else 2
        groups = ([[ics[j]] for j in range(nlead)]
                  + [ics[j:j + 2] for j in range(nlead, ni - 2, 2)]
                  + [[ics[ni - 2]], [ics[ni - 1]]])
    else:
        groups = pair_groups(range(ni), singles_at_end=2)

    nc = bacc.Bacc("TRN2", target_bir_lowering=False, debug=False,
                   num_devices=NCORES)

    xT_d = nc.dram_tensor("xT", [d, s], BF16, kind="ExternalInput").ap()
    waT_d = nc.dram_tensor("WaT", [ni, P, d], BF16, kind="ExternalInput").ap()
    wiT_d = nc.dram_tensor("WiT", [ni, P, d], BF16, kind="ExternalInput").ap()
    cst_d = nc.dram_tensor("csts", [P, len(CONST_NAMES) * ni], F32,
                           kind="ExternalInput").ap()
    out_d = nc.dram_tensor("out", [i, s], BF16, kind="ExternalOutput").ap()

    with tile.TileContext(nc) as tc:
        with ExitStack() as ctx:
            const_pool = ctx.enter_context(tc.tile_pool(name="const", bufs=1))
            xt_pool = ctx.enter_context(tc.tile_pool(name="xt", bufs=1))
            wst_pool = ctx.enter_context(tc.tile_pool(name="wst", bufs=1))
            ps_pool = ctx.enter_context(
                tc.tile_pool(name="mmpsum", bufs=1, space="PSUM"))
            rows = ctx.enter_context(tc.tile_pool(name="rows", bufs=1))

            # ---- weights for chunk 0 first (sync queue) -----------------
            w_sbs = {}

            def load_weights(ic):
                wi_sb = wst_pool.tile([P, d], BF16, name=f"wi{ic}", tag="wi",
                                      bufs=3)
                wa_sb = wst_pool.tile([P, d], BF16, name=f"wa{ic}", tag="wa",
                                      bufs=3)
                nc.sync.dma_start(wi_sb[:], wiT_d[ic])
                nc.sync.dma_start(wa_sb[:], waT_d[ic])
                w_sbs[ic] = (wi_sb, wa_sb)

            load_weights(0)

            # ---- x stream: m0 quarters from the gpsimd queue ------------
            xT_sb = [xt_pool.tile([P, s], BF16, name=f"xT{k}")
                     for k in range(nd)]
            q = 512
            for k in range(nd):
                nc.gpsimd.dma_start(xT_sb[k][:, 0:q],
                                    xT_d[k * P:(k + 1) * P, 0:q])

            # consts (one small DMA on sync queue)
            cst_t = const_pool.tile([P, len(CONST_NAMES) * ni], F32,
                                    name="cst")
            nc.sync.dma_start(cst_t[:], cst_d[:])

            def cc(nm, ic):
                base = CONST_NAMES.index(nm) * ni
                return cst_t[:, base + ic:base + ic + 1]

            # pinned program order on the ACT and PE queues: the Tile
            # scheduler otherwise interleaves silu between the tanh halves
            # and flips the pa/pi GEMM order, stalling the PE ~3.3us/chunk.
            act_chain = []

            def act(out_ap, in_ap, func, **kw):
                inst = nc.scalar.activation(out_ap, in_ap, func, **kw)
                if act_chain:
                    add_dep_helper(inst.ins, act_chain[-1].ins, False,
                                   "act order")
                act_chain.append(inst)
                return inst

            mm_chain = []

            def mm(out_ap, lhs_ap, rhs_ap, **kw):
                inst = nc.tensor.matmul(out_ap, lhs_ap, rhs_ap, **kw)
                if mm_chain:
                    add_dep_helper(inst.ins, mm_chain[-1].ins, False,
                                   "pe order")
                mm_chain.append(inst)
                return inst

            # ---- ACT table preload + PE warm-up during the DMA wait -----
            dum = rows.tile([P, 8], F32, name="dum")
            if silu:
                act(dum[:, 0:1], cc("sb", 0), AF.Silu)
            else:
                act(dum[:, 0:1], cc("sb", 0), AF.Sigmoid)
            act(dum[:, 1:2], cc("tb", 0), AF.Tanh)
            wi0 = w_sbs[0][0]
            wn = min(512, d)
            # warm-up MMs write into the pi-tag PSUM tile (reused by the
            # first real pi GEMM afterwards) so PSUM stays within 8 banks.
            warm_ps = ps_pool.tile([P, s], F32, name="warm", tag="pi",
                                   bufs=1)
            for _ in range(8 if s >= 2048 else 1):
                mm(warm_ps[:, 0:wn], wi0[:, 0:P], wi0[:, 0:wn],
                   start=True, stop=True)

            # remaining x quarters (m1..): gpsimd queue, m-major
            for m in range(1, s // q):
                for k in range(nd):
                    nc.gpsimd.dma_start(
                        xT_sb[k][:, m * q:(m + 1) * q],
                        xT_d[k * P:(k + 1) * P, m * q:(m + 1) * q])
            load_weights(1)

            def gemm(ps, w_sb):
                # m-outer, k-inner: each 512-col slice finishes ASAP
                for m in range(nmm):
                    lo = m * 512
                    for k in range(nd):
                        mm(ps[:, lo:lo + 512],
                           w_sb[:, k * P:(k + 1) * P],
                           xT_sb[k][:, lo:lo + 512],
                           start=(k == 0), stop=(k == nd - 1))

            def gemm_pa_split(ic, wa_sb):
                """pa GEMM into two [P, half] PSUM tiles (2 banks each)."""
                if nmm >= 2:
                    pa0 = ps_pool.tile([P, half], F32, name=f"pa0{ic}",
                                       tag="pa0", bufs=1)
                    pa1 = ps_pool.tile([P, half], F32, name=f"pa1{ic}",
                                       tag="pa1", bufs=1)
                    for m in range(nmm):
                        ps = pa0 if m < nmm // 2 else pa1
                        lo_t = (m % (nmm // 2)) * 512
                        lo = m * 512
                        for k in range(nd):
                            mm(ps[:, lo_t:lo_t + 512],
                               wa_sb[:, k * P:(k + 1) * P],
                               xT_sb[k][:, lo:lo + 512],
                               start=(k == 0), stop=(k == nd - 1))
                    return [pa0, pa1]
                pa0 = ps_pool.tile([P, s], F32, name=f"pa{ic}", tag="pa0",
                                   bufs=1)
                gemm(pa0, wa_sb)
                return [pa0[:, 0:half], pa0[:, half:s]]

            def chunk_front(ic, ap_t, jslot):
                """pa GEMM -> tq halves + th@full; pi GEMM -> silu@full;
                a-affine on GPSIMD. Returns (w_t, tq_t)."""
                if ic not in w_sbs:
                    load_weights(ic)
                wi_sb, wa_sb = w_sbs.pop(ic)

                # pa GEMM first, into TWO half-width PSUM tiles so the
                # h0 tanhs can start mid-GEMM (PSUM accumulation tiles
                # are dependency-tracked as one unit).
                pa_ps = gemm_pa_split(ic, wa_sb)
                tq_t = rows.tile([P, s], BF16, name=f"tq{ic}", tag="tq",
                                 bufs=3)
                th_t = rows.tile([P, s], BF16, name=f"th{ic}", tag="th",
                                 bufs=3)
                for hh in range(2):
                    sl = slice(hh * half, (hh + 1) * half)
                    act(tq_t[:, sl], pa_ps[hh][:], AF.Tanh,
                        scale=cc("qs", ic), bias=cc("qb", ic))
                    act(th_t[:, sl], pa_ps[hh][:], AF.Tanh,
                        scale=FC, bias=cc("tb", ic))

                # pi GEMM -> one 2048-wide PSUM tile -> single silu
                pi_ps = ps_pool.tile([P, s], F32, name=f"pi{ic}", tag="pi",
                                     bufs=1)
                gemm(pi_ps, wi_sb)
                w_t = rows.tile([P, s], BF16, name=f"w{ic}", tag="w", bufs=2)
                if silu:
                    act(w_t[:], pi_ps[:], AF.Silu, bias=cc("sb", ic))
                else:
                    sg = rows.tile([P, s], F32, name=f"sg{ic}", tag="sg",
                                   bufs=2)
                    act(sg[:], pi_ps[:], AF.Sigmoid, bias=cc("sb", ic))
                    pib = rows.tile([P, s], F32, name=f"pib{ic}", tag="pib",
                                    bufs=2)
                    act(pib[:], pi_ps[:], AF.Identity, bias=cc("sb", ic))
                    nc.vector.tensor_mul(w_t[:], sg[:], pib[:])

                return w_t, tq_t, th_t

            def alloc_pair(g, tag_p, tag_s, dtype, bufs_p, bufs_s):
                if len(g) == 2:
                    return rows.tile([P, 2 * s], dtype, name=f"{tag_p}{g[0]}",
                                     tag=tag_p, bufs=bufs_p)
                return rows.tile([P, s], dtype, name=f"{tag_s}{g[0]}",
                                 tag=tag_s, bufs=bufs_s)

            def scan_group(g, ap_t, cp_t, eng):
                w2 = len(g) * s
                h_t = rows.tile([P, w2], BF16, name=f"h{g[0]}",
                                tag="hp" if len(g) == 2 else "hs", bufs=2)
                eng.tensor_tensor_scan(
                    h_t[:], ap_t[:], cp_t[:], 0.0,
                    op0=ALU.mult, op1=ALU.add)
                for j, ic in enumerate(g):
                    nc.sync.dma_start(out_d[ic * P:(ic + 1) * P, :],
                                      h_t[:, j * s:(j + 1) * s])

            def last_single(ic, nparts=2):
                """Final chunk: affine+q hoisted before the pi GEMM; silu,
                c, scan, DMA at half granularity for the shortest tail."""
                if ic not in w_sbs:
                    load_weights(ic)
                wi_sb, wa_sb = w_sbs.pop(ic)
                pa_ps = gemm_pa_split(ic, wa_sb)
                tq_t = rows.tile([P, s], BF16, name=f"tq{ic}", tag="tq",
                                 bufs=3)
                th_t = rows.tile([P, s], BF16, name=f"th{ic}", tag="th",
                                 bufs=3)
                for hh in range(2):
                    sl = slice(hh * half, (hh + 1) * half)
                    act(tq_t[:, sl], pa_ps[hh][:], AF.Tanh,
                        scale=cc("qs", ic), bias=cc("qb", ic))
                    act(th_t[:, sl], pa_ps[hh][:], AF.Tanh,
                        scale=FC, bias=cc("tb", ic))
                ap_t = rows.tile([P, s], BF16, name=f"as{ic}", tag="as",
                                 bufs=2)
                nc.vector.tensor_scalar(ap_t[:], th_t[:], cc("nB", ic),
                                        cc("aA", ic), op0=ALU.mult,
                                        op1=ALU.add)
                q_t = rows.tile([P, s], BF16, name=f"q{ic}", tag="q", bufs=2)
                nc.vector.tensor_scalar(q_t[:], tq_t[:], cc("qF", ic),
                                        cc("qE", ic), op0=ALU.mult,
                                        op1=ALU.add)

                pi_ps = ps_pool.tile([P, s], F32, name=f"pi{ic}", tag="pi",
                                     bufs=1)
                gemm(pi_ps, wi_sb)
                w_t = rows.tile([P, s], BF16, name=f"w{ic}", tag="w", bufs=2)
                cp_t = rows.tile([P, s], BF16, name=f"cs{ic}", tag="cs",
                                 bufs=2)
                h_t = rows.tile([P, s], BF16, name=f"h{ic}", tag="hs",
                                bufs=2)
                qw = s // nparts
                for hh in range(nparts):
                    sl = slice(hh * qw, (hh + 1) * qw)
                    if silu:
                        act(w_t[:, sl], pi_ps[:, sl], AF.Silu,
                            bias=cc("sb", ic))
                    else:
                        sg = rows.tile([P, qw], F32, name=f"sg{ic}_{hh}",
                                       tag="sg", bufs=2)
                        act(sg[:], pi_ps[:, sl], AF.Sigmoid,
                            bias=cc("sb", ic))
                        pib = rows.tile([P, qw], F32, name=f"pib{ic}_{hh}",
                                        tag="pib", bufs=2)
                        act(pib[:], pi_ps[:, sl], AF.Identity,
                            bias=cc("sb", ic))
                        nc.vector.tensor_mul(w_t[:, sl], sg[:], pib[:])
                    nc.vector.tensor_mul(cp_t[:, sl], q_t[:, sl],
                                         w_t[:, sl])
                    nc.vector.tensor_tensor_scan(
                        h_t[:, sl], ap_t[:, sl], cp_t[:, sl],
                        0.0 if hh == 0 else h_t[:, hh * qw - 1:hh * qw],
                        op0=ALU.mult, op1=ALU.add)
                    nc.sync.dma_start(out_d[ic * P:(ic + 1) * P, sl],
                                      h_t[:, sl])

            for gi, g in enumerate(groups):
                if len(g) == 1:
                    last_single(g[0])
                    continue
                on_gp = gi in scan_gp
                ap_t = alloc_pair(g, "ap", "as", BF16, 2, 2)
                cp_t = alloc_pair(g, "cp", "cs", BF16, 2, 2)
                for j, ic in enumerate(g):
                    if on_gp:
                        aff_dve.add(ic)  # keep GPSIMD free for its scan
                    w_t, tq_t, th_t = chunk_front(ic, ap_t, j)
                    # DVE order q -> aff -> c: q's input (tq) is ready
                    # before th, so the FIFO head-blocks less
                    q_t = rows.tile([P, s], BF16, name=f"q{ic}", tag="q",
                                    bufs=2)
                    nc.vector.tensor_scalar(q_t[:], tq_t[:], cc("qF", ic),
                                            cc("qE", ic), op0=ALU.mult,
                                            op1=ALU.add)
                    a_v = ap_t[:, j * s:(j + 1) * s]
                    aff_eng = nc.vector if ic in aff_dve else nc.gpsimd
                    aff_eng.tensor_scalar(a_v, th_t[:], cc("nB", ic),
                                          cc("aA", ic), op0=ALU.mult,
                                          op1=ALU.add)
                    if j == 1:
                        nc.gpsimd.memset(ap_t[:, s:s + 1], 0.0)
                    cm_eng = nc.gpsimd if ic < CMUL_GP_N else nc.vector
                    cm_eng.tensor_mul(cp_t[:, j * s:(j + 1) * s], q_t[:],
                                      w_t[:])
                scan_group(g, ap_t, cp_t,
                           nc.gpsimd if on_gp else nc.vector)

    nc.compile()
    return nc


@functools.lru_cache(maxsize=4)
def _get_nc(s=S, d=D, i=I, nfit=0):
    return _build_nc(s, d, i, nfit=nfit)


LAST_RESULTS = None


def _prep_core_inputs(xb, shared):
    import ml_dtypes
    xT = np.ascontiguousarray(xb.T).astype(ml_dtypes.bfloat16)
    m = {"xT": xT}
    m.update(shared)
    return m


def _prep_shared(Wa, ba, Wi, bi, gate, d, i):
    """Sort channels by alpha, build device inputs. Returns
    (shared dict, nfit, perm, out_scale[i])."""
    import ml_dtypes
    ni = i // P
    nd = d // P
    alpha_u = 1.0 / (1.0 + np.exp(-gate.astype(np.float64)))
    perm = np.argsort(alpha_u, kind="stable")
    Wa = Wa[perm]
    Wi = Wi[perm]
    ba = ba[perm]
    bi = bi[perm]
    alpha = alpha_u[perm]

    WaT = np.ascontiguousarray(
        Wa.reshape(ni, P, nd, P).transpose(0, 3, 2, 1).reshape(ni, P, d)
    ).astype(ml_dtypes.bfloat16)
    WiT = np.ascontiguousarray(
        Wi.reshape(ni, P, nd, P).transpose(0, 3, 2, 1).reshape(ni, P, d)
    ).astype(ml_dtypes.bfloat16)

    aA = (alpha * FA).astype(np.float32)
    nB = (-alpha * FB).astype(np.float32)
    tb = (FC * ba.astype(np.float64) + FD).astype(np.float32)
    sb = bi.astype(np.float32)

    al = np.clip(alpha, QFIT_ALPHAS[0], QFIT_ALPHAS[-1])
    E = np.interp(al, QFIT_ALPHAS, QFIT_E)
    F = np.interp(al, QFIT_ALPHAS, QFIT_F)
    G = np.interp(al, QFIT_ALPHAS, QFIT_G)
    H = np.interp(al, QFIT_ALPHAS, QFIT_H)
    qs = G.astype(np.float32)
    qb = (G * ba.astype(np.float64) + H).astype(np.float32)
    scale = np.ones(i, np.float64)

    def vec(v):
        return np.ascontiguousarray(v.astype(np.float32).reshape(ni, P).T)

    vals = {"aA": aA, "nB": nB, "tb": tb, "sb": sb, "qs": qs, "qb": qb,
            "qE": E.astype(np.float32), "qF": F.astype(np.float32)}
    csts = np.concatenate([vec(vals[nm]) for nm in CONST_NAMES], axis=1)
    shared = {"WaT": WaT, "WiT": WiT, "csts": np.ascontiguousarray(csts)}
    return shared, ni, perm, scale.astype(np.float32)


def kernel(x, Wa, ba, Wi, bi, gate):
    global LAST_RESULTS
    from concourse.bass_utils import run_bass_kernel_spmd

    x = np.asarray(x, dtype=np.float32)
    b, s, d = x.shape
    i = Wa.shape[0]

    shared, nfit, perm, oscale = _prep_shared(
        np.asarray(Wa, np.float32), np.asarray(ba, np.float32),
        np.asarray(Wi, np.float32), np.asarray(bi, np.float32),
        np.asarray(gate, np.float32), d, i)
    nc = _get_nc(s, d, i, 0)

    in_maps = [_prep_core_inputs(x[bb], shared) for bb in range(b)]
    res = run_bass_kernel_spmd(nc, in_maps, list(range(b)))
    LAST_RESULTS = res
    out = np.empty((b, s, i), np.float32)
    for bb in range(b):
        hs = np.asarray(res.results[bb]["out"]).astype(np.float32).T * oscale
        out[bb, :, perm] = hs.T
    return out


# revision 43
# speedup vs baseline: 1.0110x; 1.0034x over previous
"""Trainium2 Bass kernel: GatedRecurrentCell (v9, all-fit).

Math (per batch b, channel i, time t):
    pa = x @ Wa^T (+ba via ACT bias) ; pi = x @ Wi^T (+bi via ACT bias)
    a  = sigmoid(gate) * 3**(-sigmoid(pa))
    c  = sqrt(1-a^2) * silu(pi)
    h_t = a_t*h_{t-1} + c_t   (h_{-1} = 0);  out = h

Design:
 1. 3**(-sigmoid(p)) == FA - FB*tanh(FC*p + FD) (5.5e-4 abs), so
    a = aA + nB*tanh(FC*pa + tb) with per-channel aA, nB.
 2. sqrt(1-a^2) ~ E + F*tanh(G*pa + H) for ALL channels (per-channel
    params from a pdf-weighted table refit; end-to-end rel err 7.8e-3,
    better than the old exact-sqrt split at 8.7e-3). This removes the
    Square/Sqrt passes entirely, so ACT uses ONE table set
    (silu_and_others: silu+tanh) -> zero table switches.
 3. q = qF*tq + qE via DVE tensor_scalar (bf16 fast mode), then
    c = q*w via DVE tensor_tensor (bf16 2x).
 4. Per chunk (PE order pinned): pa GEMM first, into TWO [128,1024]
    PSUM tiles (2 banks each) -- PSUM accumulation tiles are
    dependency-tracked as one unit, so a single 2048-wide tile forces
    every tanh to wait for the full GEMM (1.5us/chunk PE stall); the
    split lets the h0 tanhs run mid-GEMM. Then pi GEMM -> [128,2048]
    PSUM (4 banks), one silu@2048. ACT queue order pinned
    (tq_h0, th_h0, tq_h1, th_h1, silu).
 5. a-affine (a = nB*th + aA) on DVE in bf16 (tensor_scalar 4x mode,
    ~0.8us/chunk); th/a/q/c all bf16 (adds nothing visible to the
    error). GPSIMD only does seam memsets -- it otherwise contends
    with DVE for the shared SBUF port and slows fast-mode DVE ops ~3x.
 6. Recurrence: fp32 tensor_tensor_scan on DVE over pairs of chunks
    (a at the pair seam is zeroed); last 2 chunks single; the final
    chunk runs silu/c/scan/DMA at half granularity with a chained
    scan initial for the shortest tail.
 7. Startup: xT m0-quarters DMAd from the gpsimd queue in parallel
    with weights on the sync queue; dummy warm-up matmuls on wi0 warm
    the PE HAM clock-gate during the DMA wait; a dummy 1-col Silu
    triggers the ACT table load immediately (one table set total).

Mapping: data-parallel over batch (8 cores, 1 batch each); channels on
partitions (16 chunks of 128), time on the free dim. GEMMs in bf16.
"""

import functools
import os

import numpy as np

B, S, D, I = 8, 2048, 512, 2048
P = 128
NCORES = 8

# fit of 3^(-sigmoid(p)) = FA - FB*tanh(FC*p + FD), max abs err 5.5e-4
FA = 0.66661083
FB = 0.33324857
FC = 0.5096609
FD = 0.27426951

# knobs
AFF_DVE_N = int(os.environ.get("GRC_AFF_DVE", "16"))  # a-affines moved to DVE
CMUL_GP_N = int(os.environ.get("GRC_CMUL_GP", "0"))  # c=q*w on GPSIMD for
# the first N chunks (GPSIMD is otherwise idle; tail chunks stay on DVE so
# their scans aren't delayed by GPSIMD's slower elementwise rate)
SCAN_GP = os.environ.get("GRC_SCAN_GP", "")  # comma group idxs on GPSIMD

# per-alpha params of sqrt(1 - (alpha 3^-sigmoid(p))^2) ~ E + F*tanh(G*p+H)
# (pdf-weighted least-squares refit; valid for the full alpha range here)
QFIT_ALPHAS = [0.88000000, 0.88199333, 0.88398667, 0.88598000, 0.88797333, 0.88996667, 0.89196000, 0.89395333, 0.89594667, 0.89794000, 0.89993333, 0.90192667, 0.90392000, 0.90591333, 0.90790667, 0.90990000, 0.91189333, 0.91388667, 0.91588000, 0.91787333, 0.91986667, 0.92186000, 0.92385333, 0.92584667, 0.92784000, 0.92983333, 0.93182667, 0.93382000, 0.93581333, 0.93780667, 0.93980000, 0.94179333, 0.94378667, 0.94578000, 0.94777333, 0.94976667, 0.95176000, 0.95375333, 0.95574667, 0.95774000, 0.95973333, 0.96172667, 0.96372000, 0.96571333, 0.96770667, 0.96970000, 0.97169333, 0.97368667, 0.97568000, 0.97767333, 0.97966667, 0.98166000, 0.98365333, 0.98564667, 0.98764000, 0.98963333, 0.99162667, 0.99362000, 0.99561333, 0.99760667, 0.99960000]
QFIT_E = [0.71933143, 0.71742518, 0.71550108, 0.71355876, 0.71159781, 0.70961777, 0.70761823, 0.70559871, 0.70355872, 0.70149777, 0.69941532, 0.69731082, 0.69518371, 0.69303337, 0.69085916, 0.68866043, 0.6864365, 0.6841866, 0.68190999, 0.67960586, 0.67727333, 0.67491154, 0.67251952, 0.67009625, 0.6676407, 0.66515172, 0.6626281, 0.66006859, 0.65747183, 0.65483635, 0.65216063, 0.64944301, 0.64668169, 0.64387479, 0.64102025, 0.63811582, 0.63515915, 0.6321476, 0.62907833, 0.62594828, 0.62275404, 0.61949189, 0.61615776, 0.6127471, 0.60925485, 0.60567543, 0.60200249, 0.59822889, 0.59434658, 0.59034627, 0.58621727, 0.58194721, 0.57752149, 0.5729228, 0.56813033, 0.56311861, 0.55785594, 0.55230202, 0.54640405, 0.54009043, 0.53325945]
QFIT_F = [0.23774414, 0.23947298, 0.24121995, 0.24298545, 0.24476994, 0.24657388, 0.24839774, 0.25024203, 0.25210728, 0.25399401, 0.25590282, 0.25783429, 0.25978905, 0.26176774, 0.26377108, 0.26579975, 0.26785453, 0.26993621, 0.27204563, 0.27418366, 0.27635126, 0.27854937, 0.28077905, 0.28304142, 0.28533761, 0.28766887, 0.29003652, 0.29244195, 0.29488666, 0.29737224, 0.29990039, 0.30247294, 0.30509185, 0.30775923, 0.31047734, 0.31324867, 0.31607585, 0.31896179, 0.32190964, 0.32492283, 0.32800514, 0.33116073, 0.33439414, 0.33771047, 0.34111538, 0.34461515, 0.34821689, 0.35192864, 0.35575949, 0.35971991, 0.36382199, 0.36807975, 0.37250975, 0.37713169, 0.38196927, 0.38705155, 0.39241485, 0.39810538, 0.40418387, 0.41073294, 0.4178704]
QFIT_G = [0.53053654, 0.53025385, 0.52996473, 0.52966898, 0.52936639, 0.52905672, 0.52873975, 0.52841521, 0.52808285, 0.5277424, 0.52739356, 0.52703604, 0.52666952, 0.52629366, 0.5259081, 0.5255125, 0.52510644, 0.52468951, 0.52426128, 0.52382129, 0.52336904, 0.52290402, 0.52242568, 0.52193342, 0.52142663, 0.52090464, 0.52036673, 0.51981216, 0.5192401, 0.51864969, 0.51803998, 0.51740998, 0.5167586, 0.51608465, 0.51538688, 0.5146639, 0.51391421, 0.51313618, 0.512328, 0.51148774, 0.51061321, 0.50970204, 0.50875159, 0.5077589, 0.50672069, 0.50563327, 0.50449247, 0.50329355, 0.5020311, 0.50069891, 0.49928974, 0.49779515, 0.49620515, 0.49450777, 0.49268853, 0.49072957, 0.48860845, 0.48629636, 0.48375515, 0.48093245, 0.47775255]
QFIT_H = [0.6873514, 0.68859901, 0.68986176, 0.69114, 0.69243407, 0.69374437, 0.69507125, 0.69641513, 0.69777642, 0.69915554, 0.70055296, 0.70196914, 0.70340457, 0.70485975, 0.70633523, 0.70783155, 0.7093493, 0.71088909, 0.71245156, 0.71403737, 0.71564725, 0.71728191, 0.71894214, 0.72062877, 0.72234265, 0.72408469, 0.72585588, 0.72765722, 0.72948979, 0.73135477, 0.73325335, 0.73518684, 0.73715666, 0.73916427, 0.74121127, 0.74329939, 0.74543044, 0.74760642, 0.7498295, 0.75210198, 0.75442641, 0.75680556, 0.75924244, 0.76174039, 0.76430309, 0.76693459, 0.76963943, 0.77242271, 0.77529012, 0.77824814, 0.78130421, 0.78446682, 0.78774591, 0.79115314, 0.79470237, 0.79841037, 0.80229777, 0.80639043, 0.81072176, 0.81533639, 0.82029642]

CONST_NAMES = ["aA", "nB", "tb", "sb", "qs", "qb", "qE", "qF"]


def _build_nc(s, d, i, nfit=0, silu=True):
    import concourse.bacc as bacc
    import concourse.mybir as mybir
    import concourse.tile as tile
    from concourse.tile import add_dep_helper
    from contextlib import ExitStack

    F32 = mybir.dt.float32
    BF16 = mybir.dt.bfloat16
    AF = mybir.ActivationFunctionType
    ALU = mybir.AluOpType

    nd = d // P            # contraction chunks (128 rows each)
    ni = i // P            # channel chunks
    nmm = s // 512         # matmuls (N=512) per GEMM
    half = s // 2          # tanh granularity

    aff_dve = set()
    if AFF_DVE_N > 0:
        aff_dve = {ni - 1 - j for j in range(min(AFF_DVE_N, ni))}
    scan_gp = {int(t) for t in SCAN_GP.split(",") if t.strip() != ""}

    def pair_groups(ics, singles_at_end=2):
        ics = list(ics)
        nsing = singles_at_end if len(ics) >= 4 else len(ics) % 2
        body = ics[:len(ics) - nsing] if nsing else ics
        gs = [body[j:j + 2] for j in range(0, len(body), 2)]
        gs += [[ic] for ic in ics[len(ics) - nsing:]] if nsing else []
        return gs

    if ni >= 4:
        # single chunk FIRST (its half-scans start ~7us earlier, pulling
        # the whole saturated-DVE schedule forward) and single LAST (short
        # tail); pairs in between
        ics = list(range(ni))
        nlead = 6 if ni >= 12 else 2
        groups = ([[ics[j]] for j in range(nlead)]
                  + [ics[j:j + 2] for j in range(nlead, ni - 2, 2)]
                  + [[ics[ni - 2]], [ics[ni - 1]]])
    else:
        groups = pair_groups(range(ni), singles_at_end=2)

    nc = bacc.Bacc("TRN2", target_bir_lowering=False, debug=False,
                   num_devices=NCORES)

    xT_d = nc.dram_tensor("xT", [d, s], BF16, kind="ExternalInput").ap()
    waT_d = nc.dram_tensor("WaT", [ni, P, d], BF16, kind="ExternalInput").ap()
    wiT_d = nc.dram_tensor("WiT", [ni, P, d], BF16, kind="ExternalInput").ap()
    cst_d = nc.dram_tensor("csts", [P, len(CONST_NAMES) * ni], F32,
                           kind="ExternalInput").ap()
    out_d = nc.dram_tensor("out", [i, s], BF16, kind="ExternalOutput").ap()

    with tile.TileContext(nc) as tc:
        with ExitStack() as ctx:
            const_pool = ctx.enter_context(tc.tile_pool(name="const", bufs=1))
            xt_pool = ctx.enter_context(tc.tile_pool(name="xt", bufs=1))
            wst_pool = ctx.enter_context(tc.tile_pool(name="wst", bufs=1))
            ps_pool = ctx.enter_context(
                tc.tile_pool(name="mmpsum", bufs=1, space="PSUM"))
            rows = ctx.enter_context(tc.tile_pool(name="rows", bufs=1))

            # ---- weights for chunk 0 first (sync queue) -----------------
            w_sbs = {}

            def load_weights(ic):
                wi_sb = wst_pool.tile([P, d], BF16, name=f"wi{ic}", tag="wi",
                                      bufs=3)
                wa_sb = wst_pool.tile([P, d], BF16, name=f"wa{ic}", tag="wa",
                                      bufs=3)
                nc.sync.dma_start(wi_sb[:], wiT_d[ic])
                nc.sync.dma_start(wa_sb[:], waT_d[ic])
                w_sbs[ic] = (wi_sb, wa_sb)

            load_weights(0)

            # ---- x stream: m0 quarters from the gpsimd queue ------------
            xT_sb = [xt_pool.tile([P, s], BF16, name=f"xT{k}")
                     for k in range(nd)]
            q = 512
            for k in range(nd):
                nc.gpsimd.dma_start(xT_sb[k][:, 0:q],
                                    xT_d[k * P:(k + 1) * P, 0:q])

            # consts (one small DMA on sync queue)
            cst_t = const_pool.tile([P, len(CONST_NAMES) * ni], F32,
                                    name="cst")
            nc.sync.dma_start(cst_t[:], cst_d[:])

            def cc(nm, ic):
                base = CONST_NAMES.index(nm) * ni
                return cst_t[:, base + ic:base + ic + 1]

            # pinned program order on the ACT and PE queues: the Tile
            # scheduler otherwise interleaves silu between the tanh halves
            # and flips the pa/pi GEMM order, stalling the PE ~3.3us/chunk.
            act_chain = []

            def act(out_ap, in_ap, func, **kw):
                inst = nc.scalar.activation(out_ap, in_ap, func, **kw)
                if act_chain:
                    add_dep_helper(inst.ins, act_chain[-1].ins, False,
                                   "act order")
                act_chain.append(inst)
                return inst

            mm_chain = []

            def mm(out_ap, lhs_ap, rhs_ap, **kw):
                inst = nc.tensor.matmul(out_ap, lhs_ap, rhs_ap, **kw)
                if mm_chain:
                    add_dep_helper(inst.ins, mm_chain[-1].ins, False,
                                   "pe order")
                mm_chain.append(inst)
                return inst

            # ---- ACT table preload + PE warm-up during the DMA wait -----
            dum = rows.tile([P, 8], F32, name="dum")
            if silu:
                act(dum[:, 0:1], cc("sb", 0), AF.Silu)
            else:
                act(dum[:, 0:1], cc("sb", 0), AF.Sigmoid)
            act(dum[:, 1:2], cc("tb", 0), AF.Tanh)
            wi0 = w_sbs[0][0]
            wn = min(512, d)
            # warm-up MMs write into the pi-tag PSUM tile (reused by the
            # first real pi GEMM afterwards) so PSUM stays within 8 banks.
            warm_ps = ps_pool.tile([P, s], F32, name="warm", tag="pi",
                                   bufs=1)
            for _ in range(8 if s >= 2048 else 1):
                mm(warm_ps[:, 0:wn], wi0[:, 0:P], wi0[:, 0:wn],
                   start=True, stop=True)

            # remaining x quarters (m1..): gpsimd queue, m-major
            for m in range(1, s // q):
                for k in range(nd):
                    nc.gpsimd.dma_start(
                        xT_sb[k][:, m * q:(m + 1) * q],
                        xT_d[k * P:(k + 1) * P, m * q:(m + 1) * q])
            load_weights(1)

            def gemm(ps, w_sb):
                # m-outer, k-inner: each 512-col slice finishes ASAP
                for m in range(nmm):
                    lo = m * 512
                    for k in range(nd):
                        mm(ps[:, lo:lo + 512],
                           w_sb[:, k * P:(k + 1) * P],
                           xT_sb[k][:, lo:lo + 512],
                           start=(k == 0), stop=(k == nd - 1))

            def gemm_pa_split(ic, wa_sb):
                """pa GEMM into two [P, half] PSUM tiles (2 banks each)."""
                if nmm >= 2:
                    pa0 = ps_pool.tile([P, half], F32, name=f"pa0{ic}",
                                       tag="pa0", bufs=1)
                    pa1 = ps_pool.tile([P, half], F32, name=f"pa1{ic}",
                                       tag="pa1", bufs=1)
                    for m in range(nmm):
                        ps = pa0 if m < nmm // 2 else pa1
                        lo_t = (m % (nmm // 2)) * 512
                        lo = m * 512
                        for k in range(nd):
                            mm(ps[:, lo_t:lo_t + 512],
                               wa_sb[:, k * P:(k + 1) * P],
                               xT_sb[k][:, lo:lo + 512],
                               start=(k == 0), stop=(k == nd - 1))
                    return [pa0, pa1]
                pa0 = ps_pool.tile([P, s], F32, name=f"pa{ic}", tag="pa0",
                                   bufs=1)
                gemm(pa0, wa_sb)
                return [pa0[:, 0:half], pa0[:, half:s]]

            def chunk_front(ic, ap_t, jslot):
                """pa GEMM -> tq halves + th@full; pi GEMM -> silu@full;
                a-affine on GPSIMD. Returns (w_t, tq_t)."""
                if ic not in w_sbs:
                    load_weights(ic)
                wi_sb, wa_sb = w_sbs.pop(ic)

                # pa GEMM first, into TWO half-width PSUM tiles so the
                # h0 tanhs can start mid-GEMM (PSUM accumulation tiles
                # are dependency-tracked as one unit).
                pa_ps = gemm_pa_split(ic, wa_sb)
                tq_t = rows.tile([P, s], BF16, name=f"tq{ic}", tag="tq",
                                 bufs=3)
                th_t = rows.tile([P, s], BF16, name=f"th{ic}", tag="th",
                                 bufs=3)
                for hh in range(2):
                    sl = slice(hh * half, (hh + 1) * half)
                    act(tq_t[:, sl], pa_ps[hh][:], AF.Tanh,
                        scale=cc("qs", ic), bias=cc("qb", ic))
                    act(th_t[:, sl], pa_ps[hh][:], AF.Tanh,
                        scale=FC, bias=cc("tb", ic))

                # pi GEMM -> one 2048-wide PSUM tile -> single silu
                pi_ps = ps_pool.tile([P, s], F32, name=f"pi{ic}", tag="pi",
                                     bufs=1)
                gemm(pi_ps, wi_sb)
                w_t = rows.tile([P, s], BF16, name=f"w{ic}", tag="w", bufs=2)
                if silu:
                    act(w_t[:], pi_ps[:], AF.Silu, bias=cc("sb", ic))
                else:
                    sg = rows.tile([P, s], F32, name=f"sg{ic}", tag="sg",
                                   bufs=2)
                    act(sg[:], pi_ps[:], AF.Sigmoid, bias=cc("sb", ic))
                    pib = rows.tile([P, s], F32, name=f"pib{ic}", tag="pib",
                                    bufs=2)
                    act(pib[:], pi_ps[:], AF.Identity, bias=cc("sb", ic))
                    nc.vector.tensor_mul(w_t[:], sg[:], pib[:])

                return w_t, tq_t, th_t

            def alloc_pair(g, tag_p, tag_s, dtype, bufs_p, bufs_s):
                if len(g) == 2:
                    return rows.tile([P, 2 * s], dtype, name=f"{tag_p}{g[0]}",
                                     tag=tag_p, bufs=bufs_p)
                return rows.tile([P, s], dtype, name=f"{tag_s}{g[0]}",
                                 tag=tag_s, bufs=bufs_s)

            def scan_group(g, ap_t, cp_t, eng):
                w2 = len(g) * s
                h_t = rows.tile([P, w2], BF16, name=f"h{g[0]}",
                                tag="hp" if len(g) == 2 else "hs", bufs=2)
                eng.tensor_tensor_scan(
                    h_t[:], ap_t[:], cp_t[:], 0.0,
                    op0=ALU.mult, op1=ALU.add)
                for j, ic in enumerate(g):
                    nc.sync.dma_start(out_d[ic * P:(ic + 1) * P, :],
                                      h_t[:, j * s:(j + 1) * s])

            def last_single(ic, nparts=2, aff_act=False):
                """Final chunk: affine+q hoisted before the pi GEMM; silu,
                c, scan, DMA at half granularity for the shortest tail."""
                if ic not in w_sbs:
                    load_weights(ic)
                wi_sb, wa_sb = w_sbs.pop(ic)
                pa_ps = gemm_pa_split(ic, wa_sb)
                tq_t = rows.tile([P, s], BF16, name=f"tq{ic}", tag="tq",
                                 bufs=3)
                th_t = rows.tile([P, s], BF16, name=f"th{ic}", tag="th",
                                 bufs=3)
                for hh in range(2):
                    sl = slice(hh * half, (hh + 1) * half)
                    act(tq_t[:, sl], pa_ps[hh][:], AF.Tanh,
                        scale=cc("qs", ic), bias=cc("qb", ic))
                    act(th_t[:, sl], pa_ps[hh][:], AF.Tanh,
                        scale=FC, bias=cc("tb", ic))
                ap_t = rows.tile([P, s], BF16, name=f"as{ic}", tag="as",
                                 bufs=2)
                q_t = rows.tile([P, s], BF16, name=f"q{ic}", tag="q", bufs=2)
                if not aff_act:
                    nc.vector.tensor_scalar(ap_t[:], th_t[:], cc("nB", ic),
                                            cc("aA", ic), op0=ALU.mult,
                                            op1=ALU.add)
                    nc.vector.tensor_scalar(q_t[:], tq_t[:], cc("qF", ic),
                                            cc("qE", ic), op0=ALU.mult,
                                            op1=ALU.add)

                pi_ps = ps_pool.tile([P, s], F32, name=f"pi{ic}", tag="pi",
                                     bufs=1)
                gemm(pi_ps, wi_sb)
                w_t = rows.tile([P, s], BF16, name=f"w{ic}", tag="w", bufs=2)
                cp_t = rows.tile([P, s], BF16, name=f"cs{ic}", tag="cs",
                                 bufs=2)
                h_t = rows.tile([P, s], BF16, name=f"h{ic}", tag="hs",
                                bufs=2)
                qw = s // nparts
                if aff_act:
                    for hh in range(nparts):
                        sl = slice(hh * qw, (hh + 1) * qw)
                        if silu:
                            act(w_t[:, sl], pi_ps[:, sl], AF.Silu,
                                bias=cc("sb", ic))
                        else:
                            sg = rows.tile([P, qw], F32,
                                           name=f"sg{ic}_{hh}", tag="sg",
                                           bufs=2)
                            act(sg[:], pi_ps[:, sl], AF.Sigmoid,
                                bias=cc("sb", ic))
                            pib = rows.tile([P, qw], F32,
                                            name=f"pib{ic}_{hh}", tag="pib",
                                            bufs=2)
                            act(pib[:], pi_ps[:, sl], AF.Identity,
                                bias=cc("sb", ic))
                            nc.vector.tensor_mul(w_t[:, sl], sg[:], pib[:])
                    # affines on the (now idle) ACT, parallel to the DVE
                    # tail scans
                    act(ap_t[:], th_t[:], AF.Identity,
                        scale=cc("nB", ic), bias=cc("aA", ic))
                    act(q_t[:], tq_t[:], AF.Identity,
                        scale=cc("qF", ic), bias=cc("qE", ic))
                    for hh in range(nparts):
                        sl = slice(hh * qw, (hh + 1) * qw)
                        nc.vector.tensor_mul(cp_t[:, sl], q_t[:, sl],
                                             w_t[:, sl])
                        nc.vector.tensor_tensor_scan(
                            h_t[:, sl], ap_t[:, sl], cp_t[:, sl],
                            0.0 if hh == 0 else h_t[:, hh * qw - 1:hh * qw],
                            op0=ALU.mult, op1=ALU.add)
                        nc.sync.dma_start(out_d[ic * P:(ic + 1) * P, sl],
                                          h_t[:, sl])
                    return
                for hh in range(nparts):
                    sl = slice(hh * qw, (hh + 1) * qw)
                    if silu:
                        act(w_t[:, sl], pi_ps[:, sl], AF.Silu,
                            bias=cc("sb", ic))
                    else:
                        sg = rows.tile([P, qw], F32, name=f"sg{ic}_{hh}",
                                       tag="sg", bufs=2)
                        act(sg[:], pi_ps[:, sl], AF.Sigmoid,
                            bias=cc("sb", ic))
                        pib = rows.tile([P, qw], F32, name=f"pib{ic}_{hh}",
                                        tag="pib", bufs=2)
                        act(pib[:], pi_ps[:, sl], AF.Identity,
                            bias=cc("sb", ic))
                        nc.vector.tensor_mul(w_t[:, sl], sg[:], pib[:])
                    nc.vector.tensor_mul(cp_t[:, sl], q_t[:, sl],
                                         w_t[:, sl])
                    nc.vector.tensor_tensor_scan(
                        h_t[:, sl], ap_t[:, sl], cp_t[:, sl],
                        0.0 if hh == 0 else h_t[:, hh * qw - 1:hh * qw],
                        op0=ALU.mult, op1=ALU.add)
                    nc.sync.dma_start(out_d[ic * P:(ic + 1) * P, sl],
                                      h_t[:, sl])

            for gi, g in enumerate(groups):
                if len(g) == 1:
                    last_single(g[0], aff_act=(gi == len(groups) - 1))
                    continue
                on_gp = gi in scan_gp
                ap_t = alloc_pair(g, "ap", "as", BF16, 2, 2)
                cp_t = alloc_pair(g, "cp", "cs", BF16, 2, 2)
                for j, ic in enumerate(g):
                    if on_gp:
                        aff_dve.add(ic)  # keep GPSIMD free for its scan
                    w_t, tq_t, th_t = chunk_front(ic, ap_t, j)
                    # DVE order q -> aff -> c: q's input (tq) is ready
                    # before th, so the FIFO head-blocks less
                    q_t = rows.tile([P, s], BF16, name=f"q{ic}", tag="q",
                                    bufs=2)
                    nc.vector.tensor_scalar(q_t[:], tq_t[:], cc("qF", ic),
                                            cc("qE", ic), op0=ALU.mult,
                                            op1=ALU.add)
                    a_v = ap_t[:, j * s:(j + 1) * s]
                    aff_eng = nc.vector if ic in aff_dve else nc.gpsimd
                    aff_eng.tensor_scalar(a_v, th_t[:], cc("nB", ic),
                                          cc("aA", ic), op0=ALU.mult,
                                          op1=ALU.add)
                    if j == 1:
                        nc.gpsimd.memset(ap_t[:, s:s + 1], 0.0)
                    cm_eng = nc.gpsimd if ic < CMUL_GP_N else nc.vector
                    cm_eng.tensor_mul(cp_t[:, j * s:(j + 1) * s], q_t[:],
                                      w_t[:])
                scan_group(g, ap_t, cp_t,
                           nc.gpsimd if on_gp else nc.vector)

    nc.compile()
    return nc


@functools.lru_cache(maxsize=4)
def _get_nc(s=S, d=D, i=I, nfit=0):
    return _build_nc(s, d, i, nfit=nfit)


LAST_RESULTS = None


def _prep_core_inputs(xb, shared):
    import ml_dtypes
    xT = np.ascontiguousarray(xb.T).astype(ml_dtypes.bfloat16)
    m = {"xT": xT}
    m.update(shared)
    return m


def _prep_shared(Wa, ba, Wi, bi, gate, d, i):
    """Sort channels by alpha, build device inputs. Returns
    (shared dict, nfit, perm, out_scale[i])."""
    import ml_dtypes
    ni = i // P
    nd = d // P
    alpha_u = 1.0 / (1.0 + np.exp(-gate.astype(np.float64)))
    perm = np.argsort(alpha_u, kind="stable")
    Wa = Wa[perm]
    Wi = Wi[perm]
    ba = ba[perm]
    bi = bi[perm]
    alpha = alpha_u[perm]

    WaT = np.ascontiguousarray(
        Wa.reshape(ni, P, nd, P).transpose(0, 3, 2, 1).reshape(ni, P, d)
    ).astype(ml_dtypes.bfloat16)
    WiT = np.ascontiguousarray(
        Wi.reshape(ni, P, nd, P).transpose(0, 3, 2, 1).reshape(ni, P, d)
    ).astype(ml_dtypes.bfloat16)

    aA = (alpha * FA).astype(np.float32)
    nB = (-alpha * FB).astype(np.float32)
    tb = (FC * ba.astype(np.float64) + FD).astype(np.float32)
    sb = bi.astype(np.float32)

    al = np.clip(alpha, QFIT_ALPHAS[0], QFIT_ALPHAS[-1])
    E = np.interp(al, QFIT_ALPHAS, QFIT_E)
    F = np.interp(al, QFIT_ALPHAS, QFIT_F)
    G = np.interp(al, QFIT_ALPHAS, QFIT_G)
    H = np.interp(al, QFIT_ALPHAS, QFIT_H)
    qs = G.astype(np.float32)
    qb = (G * ba.astype(np.float64) + H).astype(np.float32)
    scale = np.ones(i, np.float64)

    def vec(v):
        return np.ascontiguousarray(v.astype(np.float32).reshape(ni, P).T)

    vals = {"aA": aA, "nB": nB, "tb": tb, "sb": sb, "qs": qs, "qb": qb,
            "qE": E.astype(np.float32), "qF": F.astype(np.float32)}
    csts = np.concatenate([vec(vals[nm]) for nm in CONST_NAMES], axis=1)
    shared = {"WaT": WaT, "WiT": WiT, "csts": np.ascontiguousarray(csts)}
    return shared, ni, perm, scale.astype(np.float32)


def kernel(x, Wa, ba, Wi, bi, gate):
    global LAST_RESULTS
    from concourse.bass_utils import run_bass_kernel_spmd

    x = np.asarray(x, dtype=np.float32)
    b, s, d = x.shape
    i = Wa.shape[0]

    shared, nfit, perm, oscale = _prep_shared(
        np.asarray(Wa, np.float32), np.asarray(ba, np.float32),
        np.asarray(Wi, np.float32), np.asarray(bi, np.float32),
        np.asarray(gate, np.float32), d, i)
    nc = _get_nc(s, d, i, 0)

    in_maps = [_prep_core_inputs(x[bb], shared) for bb in range(b)]
    res = run_bass_kernel_spmd(nc, in_maps, list(range(b)))
    LAST_RESULTS = res
    out = np.empty((b, s, i), np.float32)
    for bb in range(b):
        hs = np.asarray(res.results[bb]["out"]).astype(np.float32).T * oscale
        out[bb, :, perm] = hs.T
    return out


# revision 44
# speedup vs baseline: 1.0160x; 1.0049x over previous
"""Trainium2 Bass kernel: GatedRecurrentCell (v9, all-fit).

Math (per batch b, channel i, time t):
    pa = x @ Wa^T (+ba via ACT bias) ; pi = x @ Wi^T (+bi via ACT bias)
    a  = sigmoid(gate) * 3**(-sigmoid(pa))
    c  = sqrt(1-a^2) * silu(pi)
    h_t = a_t*h_{t-1} + c_t   (h_{-1} = 0);  out = h

Design:
 1. 3**(-sigmoid(p)) == FA - FB*tanh(FC*p + FD) (5.5e-4 abs), so
    a = aA + nB*tanh(FC*pa + tb) with per-channel aA, nB.
 2. sqrt(1-a^2) ~ E + F*tanh(G*pa + H) for ALL channels (per-channel
    params from a pdf-weighted table refit; end-to-end rel err 7.8e-3,
    better than the old exact-sqrt split at 8.7e-3). This removes the
    Square/Sqrt passes entirely, so ACT uses ONE table set
    (silu_and_others: silu+tanh) -> zero table switches.
 3. q = qF*tq + qE via DVE tensor_scalar (bf16 fast mode), then
    c = q*w via DVE tensor_tensor (bf16 2x).
 4. Per chunk (PE order pinned): pa GEMM first, into TWO [128,1024]
    PSUM tiles (2 banks each) -- PSUM accumulation tiles are
    dependency-tracked as one unit, so a single 2048-wide tile forces
    every tanh to wait for the full GEMM (1.5us/chunk PE stall); the
    split lets the h0 tanhs run mid-GEMM. Then pi GEMM -> [128,2048]
    PSUM (4 banks), one silu@2048. ACT queue order pinned
    (tq_h0, th_h0, tq_h1, th_h1, silu).
 5. a-affine (a = nB*th + aA) on DVE in bf16 (tensor_scalar 4x mode,
    ~0.8us/chunk); th/a/q/c all bf16 (adds nothing visible to the
    error). GPSIMD only does seam memsets -- it otherwise contends
    with DVE for the shared SBUF port and slows fast-mode DVE ops ~3x.
 6. Recurrence: fp32 tensor_tensor_scan on DVE over pairs of chunks
    (a at the pair seam is zeroed); last 2 chunks single; the final
    chunk runs silu/c/scan/DMA at half granularity with a chained
    scan initial for the shortest tail.
 7. Startup: xT m0-quarters DMAd from the gpsimd queue in parallel
    with weights on the sync queue; dummy warm-up matmuls on wi0 warm
    the PE HAM clock-gate during the DMA wait; a dummy 1-col Silu
    triggers the ACT table load immediately (one table set total).

Mapping: data-parallel over batch (8 cores, 1 batch each); channels on
partitions (16 chunks of 128), time on the free dim. GEMMs in bf16.
"""

import functools
import os

import numpy as np

B, S, D, I = 8, 2048, 512, 2048
P = 128
NCORES = 8

# fit of 3^(-sigmoid(p)) = FA - FB*tanh(FC*p + FD), max abs err 5.5e-4
FA = 0.66661083
FB = 0.33324857
FC = 0.5096609
FD = 0.27426951

# knobs
AFF_DVE_N = int(os.environ.get("GRC_AFF_DVE", "16"))  # a-affines moved to DVE
CMUL_GP_N = int(os.environ.get("GRC_CMUL_GP", "0"))  # c=q*w on GPSIMD for
# the first N chunks (GPSIMD is otherwise idle; tail chunks stay on DVE so
# their scans aren't delayed by GPSIMD's slower elementwise rate)
SCAN_GP = os.environ.get("GRC_SCAN_GP", "")  # comma group idxs on GPSIMD

# per-alpha params of sqrt(1 - (alpha 3^-sigmoid(p))^2) ~ E + F*tanh(G*p+H)
# (pdf-weighted least-squares refit; valid for the full alpha range here)
QFIT_ALPHAS = [0.88000000, 0.88199333, 0.88398667, 0.88598000, 0.88797333, 0.88996667, 0.89196000, 0.89395333, 0.89594667, 0.89794000, 0.89993333, 0.90192667, 0.90392000, 0.90591333, 0.90790667, 0.90990000, 0.91189333, 0.91388667, 0.91588000, 0.91787333, 0.91986667, 0.92186000, 0.92385333, 0.92584667, 0.92784000, 0.92983333, 0.93182667, 0.93382000, 0.93581333, 0.93780667, 0.93980000, 0.94179333, 0.94378667, 0.94578000, 0.94777333, 0.94976667, 0.95176000, 0.95375333, 0.95574667, 0.95774000, 0.95973333, 0.96172667, 0.96372000, 0.96571333, 0.96770667, 0.96970000, 0.97169333, 0.97368667, 0.97568000, 0.97767333, 0.97966667, 0.98166000, 0.98365333, 0.98564667, 0.98764000, 0.98963333, 0.99162667, 0.99362000, 0.99561333, 0.99760667, 0.99960000]
QFIT_E = [0.71933143, 0.71742518, 0.71550108, 0.71355876, 0.71159781, 0.70961777, 0.70761823, 0.70559871, 0.70355872, 0.70149777, 0.69941532, 0.69731082, 0.69518371, 0.69303337, 0.69085916, 0.68866043, 0.6864365, 0.6841866, 0.68190999, 0.67960586, 0.67727333, 0.67491154, 0.67251952, 0.67009625, 0.6676407, 0.66515172, 0.6626281, 0.66006859, 0.65747183, 0.65483635, 0.65216063, 0.64944301, 0.64668169, 0.64387479, 0.64102025, 0.63811582, 0.63515915, 0.6321476, 0.62907833, 0.62594828, 0.62275404, 0.61949189, 0.61615776, 0.6127471, 0.60925485, 0.60567543, 0.60200249, 0.59822889, 0.59434658, 0.59034627, 0.58621727, 0.58194721, 0.57752149, 0.5729228, 0.56813033, 0.56311861, 0.55785594, 0.55230202, 0.54640405, 0.54009043, 0.53325945]
QFIT_F = [0.23774414, 0.23947298, 0.24121995, 0.24298545, 0.24476994, 0.24657388, 0.24839774, 0.25024203, 0.25210728, 0.25399401, 0.25590282, 0.25783429, 0.25978905, 0.26176774, 0.26377108, 0.26579975, 0.26785453, 0.26993621, 0.27204563, 0.27418366, 0.27635126, 0.27854937, 0.28077905, 0.28304142, 0.28533761, 0.28766887, 0.29003652, 0.29244195, 0.29488666, 0.29737224, 0.29990039, 0.30247294, 0.30509185, 0.30775923, 0.31047734, 0.31324867, 0.31607585, 0.31896179, 0.32190964, 0.32492283, 0.32800514, 0.33116073, 0.33439414, 0.33771047, 0.34111538, 0.34461515, 0.34821689, 0.35192864, 0.35575949, 0.35971991, 0.36382199, 0.36807975, 0.37250975, 0.37713169, 0.38196927, 0.38705155, 0.39241485, 0.39810538, 0.40418387, 0.41073294, 0.4178704]
QFIT_G = [0.53053654, 0.53025385, 0.52996473, 0.52966898, 0.52936639, 0.52905672, 0.52873975, 0.52841521, 0.52808285, 0.5277424, 0.52739356, 0.52703604, 0.52666952, 0.52629366, 0.5259081, 0.5255125, 0.52510644, 0.52468951, 0.52426128, 0.52382129, 0.52336904, 0.52290402, 0.52242568, 0.52193342, 0.52142663, 0.52090464, 0.52036673, 0.51981216, 0.5192401, 0.51864969, 0.51803998, 0.51740998, 0.5167586, 0.51608465, 0.51538688, 0.5146639, 0.51391421, 0.51313618, 0.512328, 0.51148774, 0.51061321, 0.50970204, 0.50875159, 0.5077589, 0.50672069, 0.50563327, 0.50449247, 0.50329355, 0.5020311, 0.50069891, 0.49928974, 0.49779515, 0.49620515, 0.49450777, 0.49268853, 0.49072957, 0.48860845, 0.48629636, 0.48375515, 0.48093245, 0.47775255]
QFIT_H = [0.6873514, 0.68859901, 0.68986176, 0.69114, 0.69243407, 0.69374437, 0.69507125, 0.69641513, 0.69777642, 0.69915554, 0.70055296, 0.70196914, 0.70340457, 0.70485975, 0.70633523, 0.70783155, 0.7093493, 0.71088909, 0.71245156, 0.71403737, 0.71564725, 0.71728191, 0.71894214, 0.72062877, 0.72234265, 0.72408469, 0.72585588, 0.72765722, 0.72948979, 0.73135477, 0.73325335, 0.73518684, 0.73715666, 0.73916427, 0.74121127, 0.74329939, 0.74543044, 0.74760642, 0.7498295, 0.75210198, 0.75442641, 0.75680556, 0.75924244, 0.76174039, 0.76430309, 0.76693459, 0.76963943, 0.77242271, 0.77529012, 0.77824814, 0.78130421, 0.78446682, 0.78774591, 0.79115314, 0.79470237, 0.79841037, 0.80229777, 0.80639043, 0.81072176, 0.81533639, 0.82029642]

CONST_NAMES = ["aA", "nB", "tb", "sb", "qs", "qb", "qE", "qF"]


def _build_nc(s, d, i, nfit=0, silu=True):
    import concourse.bacc as bacc
    import concourse.mybir as mybir
    import concourse.tile as tile
    from concourse.tile import add_dep_helper
    from contextlib import ExitStack

    F32 = mybir.dt.float32
    BF16 = mybir.dt.bfloat16
    AF = mybir.ActivationFunctionType
    ALU = mybir.AluOpType

    nd = d // P            # contraction chunks (128 rows each)
    ni = i // P            # channel chunks
    nmm = s // 512         # matmuls (N=512) per GEMM
    half = s // 2          # tanh granularity

    aff_dve = set()
    if AFF_DVE_N > 0:
        aff_dve = {ni - 1 - j for j in range(min(AFF_DVE_N, ni))}
    scan_gp = {int(t) for t in SCAN_GP.split(",") if t.strip() != ""}

    def pair_groups(ics, singles_at_end=2):
        ics = list(ics)
        nsing = singles_at_end if len(ics) >= 4 else len(ics) % 2
        body = ics[:len(ics) - nsing] if nsing else ics
        gs = [body[j:j + 2] for j in range(0, len(body), 2)]
        gs += [[ic] for ic in ics[len(ics) - nsing:]] if nsing else []
        return gs

    if ni >= 4:
        # single chunk FIRST (its half-scans start ~7us earlier, pulling
        # the whole saturated-DVE schedule forward) and single LAST (short
        # tail); pairs in between
        ics = list(range(ni))
        nlead = 6 if ni >= 12 else 2
        groups = ([[ics[j]] for j in range(nlead)]
                  + [ics[j:j + 2] for j in range(nlead, ni - 2, 2)]
                  + [[ics[ni - 2]], [ics[ni - 1]]])
    else:
        groups = pair_groups(range(ni), singles_at_end=2)

    nc = bacc.Bacc("TRN2", target_bir_lowering=False, debug=False,
                   num_devices=NCORES)

    xT_d = nc.dram_tensor("xT", [d, s], BF16, kind="ExternalInput").ap()
    waT_d = nc.dram_tensor("WaT", [ni, P, d], BF16, kind="ExternalInput").ap()
    wiT_d = nc.dram_tensor("WiT", [ni, P, d], BF16, kind="ExternalInput").ap()
    cst_d = nc.dram_tensor("csts", [P, len(CONST_NAMES) * ni], F32,
                           kind="ExternalInput").ap()
    out_d = nc.dram_tensor("out", [i, s], BF16, kind="ExternalOutput").ap()

    with tile.TileContext(nc) as tc:
        with ExitStack() as ctx:
            const_pool = ctx.enter_context(tc.tile_pool(name="const", bufs=1))
            xt_pool = ctx.enter_context(tc.tile_pool(name="xt", bufs=1))
            wst_pool = ctx.enter_context(tc.tile_pool(name="wst", bufs=1))
            ps_pool = ctx.enter_context(
                tc.tile_pool(name="mmpsum", bufs=1, space="PSUM"))
            rows = ctx.enter_context(tc.tile_pool(name="rows", bufs=1))

            # ---- weights for chunk 0 first (sync queue) -----------------
            w_sbs = {}

            def load_weights(ic):
                wi_sb = wst_pool.tile([P, d], BF16, name=f"wi{ic}", tag="wi",
                                      bufs=3)
                wa_sb = wst_pool.tile([P, d], BF16, name=f"wa{ic}", tag="wa",
                                      bufs=3)
                nc.sync.dma_start(wi_sb[:], wiT_d[ic])
                nc.sync.dma_start(wa_sb[:], waT_d[ic])
                w_sbs[ic] = (wi_sb, wa_sb)

            load_weights(0)

            # ---- x stream: m0 quarters from the gpsimd queue ------------
            xT_sb = [xt_pool.tile([P, s], BF16, name=f"xT{k}")
                     for k in range(nd)]
            q = 512
            for k in range(nd):
                nc.gpsimd.dma_start(xT_sb[k][:, 0:q],
                                    xT_d[k * P:(k + 1) * P, 0:q])

            # consts (one small DMA on sync queue)
            cst_t = const_pool.tile([P, len(CONST_NAMES) * ni], F32,
                                    name="cst")
            nc.sync.dma_start(cst_t[:], cst_d[:])

            def cc(nm, ic):
                base = CONST_NAMES.index(nm) * ni
                return cst_t[:, base + ic:base + ic + 1]

            # pinned program order on the ACT and PE queues: the Tile
            # scheduler otherwise interleaves silu between the tanh halves
            # and flips the pa/pi GEMM order, stalling the PE ~3.3us/chunk.
            act_chain = []

            def act(out_ap, in_ap, func, **kw):
                inst = nc.scalar.activation(out_ap, in_ap, func, **kw)
                if act_chain:
                    add_dep_helper(inst.ins, act_chain[-1].ins, False,
                                   "act order")
                act_chain.append(inst)
                return inst

            mm_chain = []

            def mm(out_ap, lhs_ap, rhs_ap, **kw):
                inst = nc.tensor.matmul(out_ap, lhs_ap, rhs_ap, **kw)
                if mm_chain:
                    add_dep_helper(inst.ins, mm_chain[-1].ins, False,
                                   "pe order")
                mm_chain.append(inst)
                return inst

            # ---- ACT table preload + PE warm-up during the DMA wait -----
            dum = rows.tile([P, 8], F32, name="dum")
            if silu:
                act(dum[:, 0:1], cc("sb", 0), AF.Silu)
            else:
                act(dum[:, 0:1], cc("sb", 0), AF.Sigmoid)
            act(dum[:, 1:2], cc("tb", 0), AF.Tanh)
            wi0 = w_sbs[0][0]
            wn = min(512, d)
            # warm-up MMs write into the pi-tag PSUM tile (reused by the
            # first real pi GEMM afterwards) so PSUM stays within 8 banks.
            warm_ps = ps_pool.tile([P, s], F32, name="warm", tag="pi",
                                   bufs=1)
            for _ in range(8 if s >= 2048 else 1):
                mm(warm_ps[:, 0:wn], wi0[:, 0:P], wi0[:, 0:wn],
                   start=True, stop=True)

            # remaining x quarters (m1..): gpsimd queue, m-major
            for m in range(1, s // q):
                for k in range(nd):
                    nc.gpsimd.dma_start(
                        xT_sb[k][:, m * q:(m + 1) * q],
                        xT_d[k * P:(k + 1) * P, m * q:(m + 1) * q])
            load_weights(1)

            def gemm(ps, w_sb):
                # m-outer, k-inner: each 512-col slice finishes ASAP
                for m in range(nmm):
                    lo = m * 512
                    for k in range(nd):
                        mm(ps[:, lo:lo + 512],
                           w_sb[:, k * P:(k + 1) * P],
                           xT_sb[k][:, lo:lo + 512],
                           start=(k == 0), stop=(k == nd - 1))

            def gemm_pa_split(ic, wa_sb):
                """pa GEMM into two [P, half] PSUM tiles (2 banks each)."""
                if nmm >= 2:
                    pa0 = ps_pool.tile([P, half], F32, name=f"pa0{ic}",
                                       tag="pa0", bufs=1)
                    pa1 = ps_pool.tile([P, half], F32, name=f"pa1{ic}",
                                       tag="pa1", bufs=1)
                    for m in range(nmm):
                        ps = pa0 if m < nmm // 2 else pa1
                        lo_t = (m % (nmm // 2)) * 512
                        lo = m * 512
                        for k in range(nd):
                            mm(ps[:, lo_t:lo_t + 512],
                               wa_sb[:, k * P:(k + 1) * P],
                               xT_sb[k][:, lo:lo + 512],
                               start=(k == 0), stop=(k == nd - 1))
                    return [pa0, pa1]
                pa0 = ps_pool.tile([P, s], F32, name=f"pa{ic}", tag="pa0",
                                   bufs=1)
                gemm(pa0, wa_sb)
                return [pa0[:, 0:half], pa0[:, half:s]]

            def chunk_front(ic, ap_t, jslot):
                """pa GEMM -> tq halves + th@full; pi GEMM -> silu@full;
                a-affine on GPSIMD. Returns (w_t, tq_t)."""
                if ic not in w_sbs:
                    load_weights(ic)
                wi_sb, wa_sb = w_sbs.pop(ic)

                # pa GEMM first, into TWO half-width PSUM tiles so the
                # h0 tanhs can start mid-GEMM (PSUM accumulation tiles
                # are dependency-tracked as one unit).
                pa_ps = gemm_pa_split(ic, wa_sb)
                tq_t = rows.tile([P, s], BF16, name=f"tq{ic}", tag="tq",
                                 bufs=3)
                th_t = rows.tile([P, s], BF16, name=f"th{ic}", tag="th",
                                 bufs=3)
                for hh in range(2):
                    sl = slice(hh * half, (hh + 1) * half)
                    act(tq_t[:, sl], pa_ps[hh][:], AF.Tanh,
                        scale=cc("qs", ic), bias=cc("qb", ic))
                    act(th_t[:, sl], pa_ps[hh][:], AF.Tanh,
                        scale=FC, bias=cc("tb", ic))

                # pi GEMM -> one 2048-wide PSUM tile -> single silu
                pi_ps = ps_pool.tile([P, s], F32, name=f"pi{ic}", tag="pi",
                                     bufs=1)
                gemm(pi_ps, wi_sb)
                w_t = rows.tile([P, s], BF16, name=f"w{ic}", tag="w", bufs=2)
                if silu:
                    act(w_t[:], pi_ps[:], AF.Silu, bias=cc("sb", ic))
                else:
                    sg = rows.tile([P, s], F32, name=f"sg{ic}", tag="sg",
                                   bufs=2)
                    act(sg[:], pi_ps[:], AF.Sigmoid, bias=cc("sb", ic))
                    pib = rows.tile([P, s], F32, name=f"pib{ic}", tag="pib",
                                    bufs=2)
                    act(pib[:], pi_ps[:], AF.Identity, bias=cc("sb", ic))
                    nc.vector.tensor_mul(w_t[:], sg[:], pib[:])

                return w_t, tq_t, th_t

            def alloc_pair(g, tag_p, tag_s, dtype, bufs_p, bufs_s):
                if len(g) == 2:
                    return rows.tile([P, 2 * s], dtype, name=f"{tag_p}{g[0]}",
                                     tag=tag_p, bufs=bufs_p)
                return rows.tile([P, s], dtype, name=f"{tag_s}{g[0]}",
                                 tag=tag_s, bufs=bufs_s)

            def scan_group(g, ap_t, cp_t, eng):
                w2 = len(g) * s
                h_t = rows.tile([P, w2], BF16, name=f"h{g[0]}",
                                tag="hp" if len(g) == 2 else "hs", bufs=2)
                eng.tensor_tensor_scan(
                    h_t[:], ap_t[:], cp_t[:], 0.0,
                    op0=ALU.mult, op1=ALU.add)
                for j, ic in enumerate(g):
                    nc.sync.dma_start(out_d[ic * P:(ic + 1) * P, :],
                                      h_t[:, j * s:(j + 1) * s])

            def last_single(ic, nparts=2, aff_act=False):
                """Final chunk: affine+q hoisted before the pi GEMM; silu,
                c, scan, DMA at half granularity for the shortest tail."""
                if ic not in w_sbs:
                    load_weights(ic)
                wi_sb, wa_sb = w_sbs.pop(ic)
                pa_ps = gemm_pa_split(ic, wa_sb)
                tq_t = rows.tile([P, s], BF16, name=f"tq{ic}", tag="tq",
                                 bufs=3)
                th_t = rows.tile([P, s], BF16, name=f"th{ic}", tag="th",
                                 bufs=3)
                for hh in range(2):
                    sl = slice(hh * half, (hh + 1) * half)
                    act(tq_t[:, sl], pa_ps[hh][:], AF.Tanh,
                        scale=cc("qs", ic), bias=cc("qb", ic))
                    act(th_t[:, sl], pa_ps[hh][:], AF.Tanh,
                        scale=FC, bias=cc("tb", ic))
                ap_t = rows.tile([P, s], BF16, name=f"as{ic}", tag="as",
                                 bufs=2)
                q_t = rows.tile([P, s], BF16, name=f"q{ic}", tag="q", bufs=2)
                nc.vector.tensor_scalar(ap_t[:], th_t[:], cc("nB", ic),
                                        cc("aA", ic), op0=ALU.mult,
                                        op1=ALU.add)
                if not aff_act:
                    nc.vector.tensor_scalar(q_t[:], tq_t[:], cc("qF", ic),
                                            cc("qE", ic), op0=ALU.mult,
                                            op1=ALU.add)

                pi_ps = ps_pool.tile([P, s], F32, name=f"pi{ic}", tag="pi",
                                     bufs=1)
                gemm(pi_ps, wi_sb)
                w_t = rows.tile([P, s], BF16, name=f"w{ic}", tag="w", bufs=2)
                cp_t = rows.tile([P, s], BF16, name=f"cs{ic}", tag="cs",
                                 bufs=2)
                h_t = rows.tile([P, s], BF16, name=f"h{ic}", tag="hs",
                                bufs=2)
                qw = s // nparts
                if aff_act:
                    for hh in range(nparts):
                        sl = slice(hh * qw, (hh + 1) * qw)
                        if silu:
                            act(w_t[:, sl], pi_ps[:, sl], AF.Silu,
                                bias=cc("sb", ic))
                        else:
                            sg = rows.tile([P, qw], F32,
                                           name=f"sg{ic}_{hh}", tag="sg",
                                           bufs=2)
                            act(sg[:], pi_ps[:, sl], AF.Sigmoid,
                                bias=cc("sb", ic))
                            pib = rows.tile([P, qw], F32,
                                            name=f"pib{ic}_{hh}", tag="pib",
                                            bufs=2)
                            act(pib[:], pi_ps[:, sl], AF.Identity,
                                bias=cc("sb", ic))
                            nc.vector.tensor_mul(w_t[:, sl], sg[:], pib[:])
                    # q on the (now idle) ACT, parallel to the DVE tail
                    # scans; the affine stays on DVE (hoisted, runs early)
                    act(q_t[:], tq_t[:], AF.Identity,
                        scale=cc("qF", ic), bias=cc("qE", ic))
                    for hh in range(nparts):
                        sl = slice(hh * qw, (hh + 1) * qw)
                        nc.vector.tensor_mul(cp_t[:, sl], q_t[:, sl],
                                             w_t[:, sl])
                        nc.vector.tensor_tensor_scan(
                            h_t[:, sl], ap_t[:, sl], cp_t[:, sl],
                            0.0 if hh == 0 else h_t[:, hh * qw - 1:hh * qw],
                            op0=ALU.mult, op1=ALU.add)
                        nc.sync.dma_start(out_d[ic * P:(ic + 1) * P, sl],
                                          h_t[:, sl])
                    return
                for hh in range(nparts):
                    sl = slice(hh * qw, (hh + 1) * qw)
                    if silu:
                        act(w_t[:, sl], pi_ps[:, sl], AF.Silu,
                            bias=cc("sb", ic))
                    else:
                        sg = rows.tile([P, qw], F32, name=f"sg{ic}_{hh}",
                                       tag="sg", bufs=2)
                        act(sg[:], pi_ps[:, sl], AF.Sigmoid,
                            bias=cc("sb", ic))
                        pib = rows.tile([P, qw], F32, name=f"pib{ic}_{hh}",
                                        tag="pib", bufs=2)
                        act(pib[:], pi_ps[:, sl], AF.Identity,
                            bias=cc("sb", ic))
                        nc.vector.tensor_mul(w_t[:, sl], sg[:], pib[:])
                    nc.vector.tensor_mul(cp_t[:, sl], q_t[:, sl],
                                         w_t[:, sl])
                    nc.vector.tensor_tensor_scan(
                        h_t[:, sl], ap_t[:, sl], cp_t[:, sl],
                        0.0 if hh == 0 else h_t[:, hh * qw - 1:hh * qw],
                        op0=ALU.mult, op1=ALU.add)
                    nc.sync.dma_start(out_d[ic * P:(ic + 1) * P, sl],
                                      h_t[:, sl])

            for gi, g in enumerate(groups):
                if len(g) == 1:
                    last_single(g[0], aff_act=(gi == len(groups) - 1))
                    continue
                on_gp = gi in scan_gp
                ap_t = alloc_pair(g, "ap", "as", BF16, 2, 2)
                cp_t = alloc_pair(g, "cp", "cs", BF16, 2, 2)
                for j, ic in enumerate(g):
                    if on_gp:
                        aff_dve.add(ic)  # keep GPSIMD free for its scan
                    w_t, tq_t, th_t = chunk_front(ic, ap_t, j)
                    # DVE order q -> aff -> c: q's input (tq) is ready
                    # before th, so the FIFO head-blocks less
                    q_t = rows.tile([P, s], BF16, name=f"q{ic}", tag="q",
                                    bufs=2)
                    nc.vector.tensor_scalar(q_t[:], tq_t[:], cc("qF", ic),
                                            cc("qE", ic), op0=ALU.mult,
                                            op1=ALU.add)
                    a_v = ap_t[:, j * s:(j + 1) * s]
                    aff_eng = nc.vector if ic in aff_dve else nc.gpsimd
                    aff_eng.tensor_scalar(a_v, th_t[:], cc("nB", ic),
                                          cc("aA", ic), op0=ALU.mult,
                                          op1=ALU.add)
                    if j == 1:
                        nc.gpsimd.memset(ap_t[:, s:s + 1], 0.0)
                    cm_eng = nc.gpsimd if ic < CMUL_GP_N else nc.vector
                    cm_eng.tensor_mul(cp_t[:, j * s:(j + 1) * s], q_t[:],
                                      w_t[:])
                scan_group(g, ap_t, cp_t,
                           nc.gpsimd if on_gp else nc.vector)

    nc.compile()
    return nc


@functools.lru_cache(maxsize=4)
def _get_nc(s=S, d=D, i=I, nfit=0):
    return _build_nc(s, d, i, nfit=nfit)


LAST_RESULTS = None


def _prep_core_inputs(xb, shared):
    import ml_dtypes
    xT = np.ascontiguousarray(xb.T).astype(ml_dtypes.bfloat16)
    m = {"xT": xT}
    m.update(shared)
    return m


def _prep_shared(Wa, ba, Wi, bi, gate, d, i):
    """Sort channels by alpha, build device inputs. Returns
    (shared dict, nfit, perm, out_scale[i])."""
    import ml_dtypes
    ni = i // P
    nd = d // P
    alpha_u = 1.0 / (1.0 + np.exp(-gate.astype(np.float64)))
    perm = np.argsort(alpha_u, kind="stable")
    Wa = Wa[perm]
    Wi = Wi[perm]
    ba = ba[perm]
    bi = bi[perm]
    alpha = alpha_u[perm]

    WaT = np.ascontiguousarray(
        Wa.reshape(ni, P, nd, P).transpose(0, 3, 2, 1).reshape(ni, P, d)
    ).astype(ml_dtypes.bfloat16)
    WiT = np.ascontiguousarray(
        Wi.reshape(ni, P, nd, P).transpose(0, 3, 2, 1).reshape(ni, P, d)
    ).astype(ml_dtypes.bfloat16)

    aA = (alpha * FA).astype(np.float32)
    nB = (-alpha * FB).astype(np.float32)
    tb = (FC * ba.astype(np.float64) + FD).astype(np.float32)
    sb = bi.astype(np.float32)

    al = np.clip(alpha, QFIT_ALPHAS[0], QFIT_ALPHAS[-1])
    E = np.interp(al, QFIT_ALPHAS, QFIT_E)
    F = np.interp(al, QFIT_ALPHAS, QFIT_F)
    G = np.interp(al, QFIT_ALPHAS, QFIT_G)
    H = np.interp(al, QFIT_ALPHAS, QFIT_H)
    qs = G.astype(np.float32)
    qb = (G * ba.astype(np.float64) + H).astype(np.float32)
    scale = np.ones(i, np.float64)

    def vec(v):
        return np.ascontiguousarray(v.astype(np.float32).reshape(ni, P).T)

    vals = {"aA": aA, "nB": nB, "tb": tb, "sb": sb, "qs": qs, "qb": qb,
            "qE": E.astype(np.float32), "qF": F.astype(np.float32)}
    csts = np.concatenate([vec(vals[nm]) for nm in CONST_NAMES], axis=1)
    shared = {"WaT": WaT, "WiT": WiT, "csts": np.ascontiguousarray(csts)}
    return shared, ni, perm, scale.astype(np.float32)


def kernel(x, Wa, ba, Wi, bi, gate):
    global LAST_RESULTS
    from concourse.bass_utils import run_bass_kernel_spmd

    x = np.asarray(x, dtype=np.float32)
    b, s, d = x.shape
    i = Wa.shape[0]

    shared, nfit, perm, oscale = _prep_shared(
        np.asarray(Wa, np.float32), np.asarray(ba, np.float32),
        np.asarray(Wi, np.float32), np.asarray(bi, np.float32),
        np.asarray(gate, np.float32), d, i)
    nc = _get_nc(s, d, i, 0)

    in_maps = [_prep_core_inputs(x[bb], shared) for bb in range(b)]
    res = run_bass_kernel_spmd(nc, in_maps, list(range(b)))
    LAST_RESULTS = res
    out = np.empty((b, s, i), np.float32)
    for bb in range(b):
        hs = np.asarray(res.results[bb]["out"]).astype(np.float32).T * oscale
        out[bb, :, perm] = hs.T
    return out
